# revision 19
# baseline (speedup 1.0000x reference)
"""FAVOR+ attention (Performer) Trainium2 kernel, 8-way sharded.

Sharding: 8 cores = 4 batches x 2 head-groups. Core c handles batch c//2 and
heads [8*(c%2), 8*(c%2)+8). The attention core (kv state) is fully local per
head; the output projection is computed as a per-core partial over its 512
input channels and the two partials per batch are summed on the host.

All matmuls run in bf16 (1 cycle/row on the PE); accumulation is fp32 in
PSUM. The output is bias-dominated, so bf16 operand rounding keeps the final
relative error at the few-1e-3 level.
"""

import numpy as np
import ml_dtypes

import concourse.bass as bass
import concourse.mybir as mybir
import concourse.tile as tile
from concourse import bacc
from concourse.bass_utils import run_bass_kernel_spmd

F32 = mybir.dt.float32
BF16 = mybir.dt.bfloat16
AF = mybir.ActivationFunctionType
ALU = mybir.AluOpType

N = 4096
D = 1024
HD = 64
NF = 64
EPS = 1e-4
BLK = 512  # n-block
NBLK = N // BLK
NCH = BLK // 128  # 128-row chunks per block
SCALE = float(HD) ** -0.25


def _build_nc():
    nc = bacc.Bacc("TRN2", target_bir_lowering=False, debug=False, num_devices=8)

    xt = nc.dram_tensor("xt", [D, N], BF16, kind="ExternalInput").ap()
    wq = nc.dram_tensor("wq", [D, 512], BF16, kind="ExternalInput").ap()
    wk = nc.dram_tensor("wk", [D, 512], BF16, kind="ExternalInput").ap()
    wv = nc.dram_tensor("wv", [D, 512], BF16, kind="ExternalInput").ap()
    wp = nc.dram_tensor("wp", [512, D], BF16, kind="ExternalInput").ap()
    bq = nc.dram_tensor("bq", [128, 4], F32, kind="ExternalInput").ap()
    bk = nc.dram_tensor("bk", [128, 4], F32, kind="ExternalInput").ap()
    bvb = nc.dram_tensor("bvb", [128, 4, 64], F32, kind="ExternalInput").ap()
    pbdh = nc.dram_tensor("pbdh", [128, 128], BF16, kind="ExternalInput").ap()
    oh2 = nc.dram_tensor("oh2", [2, 128], BF16, kind="ExternalInput").ap()
    out = nc.dram_tensor("out", [D, N], F32, kind="ExternalOutput").ap()

    xt_v = xt.rearrange("(dc p) n -> p dc n", p=128)  # [128, 8, 4096]
    wq_v = wq.rearrange("(dc p) j -> p dc j", p=128)  # [128, 8, 512]
    wk_v = wk.rearrange("(dc p) j -> p dc j", p=128)
    wv_v = wv.rearrange("(dc p) j -> p dc j", p=128)
    wp_v = wp.rearrange("(jc p) o -> p jc o", p=128)  # [128, 4, 1024]
    out_v = out.rearrange("(oc p) n -> p oc n", p=128)  # [128, 8, 4096]

    with tile.TileContext(nc) as tc:
        with (
            tc.tile_pool(name="consts", bufs=1) as consts,
            tc.tile_pool(name="xp", bufs=3) as xp,
            tc.tile_pool(name="work", bufs=2) as work,
            tc.tile_pool(name="small", bufs=4) as small,
            tc.tile_pool(name="pbig", bufs=4, space="PSUM") as pbig,
            tc.tile_pool(name="pnrm", bufs=2, space="PSUM") as pnrm,
        ):
            pkv = tc.alloc_tile_pool(name="pkv", bufs=1, space="PSUM")
            # ---- constants / weights (pass-A-critical loads first) ----
            wk_sb = consts.tile([128, 8, 512], BF16, name="wk_sb")
            nc.scalar.dma_start(wk_sb[:, :, 0:128], wk_v[:, :, 0:128])
            pbdh_sb = consts.tile([128, 128], BF16, name="pbdh_sb")
            nc.scalar.dma_start(pbdh_sb[:], pbdh)
            bk_sb = consts.tile([128, 4], F32, name="bk_sb")
            nc.scalar.dma_start(bk_sb[:], bk)
            nc.scalar.dma_start(wk_sb[:, :, 128:512], wk_v[:, :, 128:512])
            wv_sb = consts.tile([128, 8, 512], BF16, name="wv_sb")
            nc.scalar.dma_start(wv_sb[:], wv_v)
            eps_sb = consts.tile([128, 1], F32, name="eps_sb")
            nc.vector.memset(eps_sb[:], EPS)

            # declared now, loaded during pass A
            wq_sb = consts.tile([128, 8, 512], BF16, name="wq_sb")
            wp_sb = consts.tile([128, 4, 1024], BF16, name="wp_sb")
            oh2_sb = consts.tile([2, 128], BF16, name="oh2_sb")
            bq_sb = consts.tile([128, 4], F32, name="bq_sb")
            bvb_sb = consts.tile([128, 4, 64], F32, name="bvb_sb")

            # kv accumulators: pairs (0,1) in kvacc0, (2,3) in kvacc1.
            # Layout per pair: 129 cols (64 v-head0 | 64 v-head1 | ksum), stride 130.
            kvacc = [
                pkv.tile([128, 260], F32, name=f"kvacc{t}", tag=f"kvacc{t}")
                for t in range(2)
            ]

            # ================= pass A: k', v -> kv, ksum =================
            for blk in range(NBLK):
                ns = slice(blk * BLK, (blk + 1) * BLK)
                xt_t = xp.tile([128, 8, BLK], BF16, name="xt_t", tag="xt")
                if blk == 0:
                    nc.sync.dma_start(xt_t[:, 0:4, :], xt_v[:, 0:4, ns])
                    nc.sync.dma_start(xt_t[:, 4:8, :], xt_v[:, 4:8, ns])
                else:
                    nc.sync.dma_start(xt_t[:], xt_v[:, :, ns])

                # kT [j, n] for this block (4 j-chunks = 4 head pairs)
                kt_sb = work.tile([128, 4, BLK], BF16, name="kt_sb", tag="kt")
                for jc in range(4):
                    ps = pbig.tile([128, BLK], F32, name="ps_kt", tag="big")
                    for dc in range(8):
                        nc.tensor.matmul(
                            ps[:],
                            wk_sb[:, dc, jc * 128 : (jc + 1) * 128],
                            xt_t[:, dc, :],
                            start=(dc == 0),
                            stop=(dc == 7),
                        )
                    nc.scalar.activation(
                        kt_sb[:, jc, :], ps[:], AF.Identity,
                        bias=bk_sb[:, jc : jc + 1], scale=1.0,
                    )

                # v and k-features interleaved per chunk (spreads
                # PSUM-slot consumer load between multi-MM v groups)
                v_sbs = []
                kp_sbs = []
                for c in range(NCH):
                    cs = slice(c * 128, (c + 1) * 128)
                    psv = pbig.tile([128, 512], F32, name="ps_v", tag="big")
                    for dc in range(8):
                        nc.tensor.matmul(
                            psv[:],
                            xt_t[:, dc, cs],
                            wv_sb[:, dc, :],
                            start=(dc == 0),
                            stop=(dc == 7),
                        )
                    v_sb = work.tile([128, 4, 132], BF16, name="v_sb", tag="v", bufs=5)
                    nc.scalar.copy(
                        v_sb[:, :, 0:128],
                        psv.rearrange("p (g j) -> p g j", j=128),
                    )
                    nc.vector.memset(v_sb[:, :, 128:129], 1.0)
                    v_sbs.append(v_sb)

                    psf = pbig.tile([128, 512], F32, name="ps_kf", tag="big")
                    for p in range(4):
                        nc.tensor.matmul(
                            psf[:, p * 128 : (p + 1) * 128],
                            kt_sb[:, p, cs],
                            pbdh_sb[:],
                            start=(p == 0),
                            stop=(p == 3),
                        )
                    psf_v = psf.rearrange("p (g f) -> p g f", f=64)  # [128, 8, 64]
                    mx = small.tile([128, 8], F32, name="mx", tag="mx")
                    nc.vector.reduce_max(mx[:], psf_v, axis=mybir.AxisListType.X)
                    karg = small.tile([128, 8, 64], F32, name="karg", tag="karg")
                    nc.vector.tensor_tensor(
                        karg[:], psf_v,
                        mx[:, :, None].to_broadcast([128, 8, 64]),
                        ALU.subtract,
                    )
                    kp_sb = work.tile([128, 4, 128], BF16, name="kp_sb", tag="kp", bufs=5)
                    nc.scalar.activation(
                        kp_sb.rearrange("p g (h f) -> p (g h) f", f=64),
                        karg[:], AF.Exp, bias=eps_sb[:], scale=1.0,
                    )
                    kp_sbs.append(kp_sb)

                # kv (+ksum) accumulation
                for c in range(NCH):
                    glob_first = blk == 0 and c == 0
                    glob_last = blk == NBLK - 1 and c == NCH - 1
                    for p in range(4):
                        base = (p % 2) * 130
                        nc.tensor.matmul(
                            kvacc[p // 2][:, base : base + 129],
                            kp_sbs[c][:, p, :],
                            v_sbs[c][:, p, 0:129],
                            start=(glob_first and p % 2 == 0),
                            stop=(glob_last and p % 2 == 1),
                        )

                if blk == 0:
                    # stream pass-B weights while pass A computes (gpsimd
                    # SWDGE queue; the sync queue keeps feeding xt blocks)
                    nc.gpsimd.dma_start(wq_sb[:], wq_v)
                    nc.gpsimd.dma_start(wp_sb[:], wp_v)
                    nc.gpsimd.dma_start(oh2_sb[:], oh2)
                    nc.gpsimd.dma_start(bq_sb[:], bq)
                    nc.gpsimd.dma_start(bvb_sb[:], bvb)

            # ============ assemble kv blockdiag + ksum columns ============
            kvbd = consts.tile([128, 4, 128], BF16, name="kvbd")
            ksbc = consts.tile([128, 4, 2], BF16, name="ksbc")
            nc.vector.memset(kvbd[:], 0.0)
            nc.vector.memset(ksbc[:], 0.0)
            for p in range(4):
                t = kvacc[p // 2]
                base = (p % 2) * 130
                ks = t[:, base + 128 : base + 129]
                nc.vector.tensor_copy(out=ksbc[0:64, p, 0:1], in_=ks[0:64])
                nc.vector.tensor_copy(out=ksbc[64:128, p, 1:2], in_=ks[64:128])
                # kv[h] += ksum[h] (x) bv[h], fold v-bias into kv
                nc.vector.scalar_tensor_tensor(
                    out=kvbd[0:64, p, 0:64],
                    in0=bvb_sb[0:64, p, :],
                    scalar=ks[0:64],
                    in1=t[0:64, base : base + 64],
                    op0=ALU.mult,
                    op1=ALU.add,
                )
                nc.vector.scalar_tensor_tensor(
                    out=kvbd[64:128, p, 64:128],
                    in0=bvb_sb[64:128, p, :],
                    scalar=ks[64:128],
                    in1=t[64:128, base + 64 : base + 128],
                    op0=ALU.mult,
                    op1=ALU.add,
                )

            # kv accumulator banks are dead now; hand them to pass B's
            # bc/po single-matmul stages so they don't churn the main ring.
            pkv.release()
            pmid = tc.alloc_tile_pool(name="pmid", bufs=2, space="PSUM")

            # ================= pass B: q', out, proj =================
            # The per-pair chain qf -> nrm -> bc -> po has an ACT/DVE hop
            # between every matmul stage. Interleave each stage with one qt
            # matmul group of the NEXT block so the PE never idles (and so
            # HAM keeps the PE clock at 2.4 GHz).
            def emit_qt_start(blk):
                ns = slice(blk * BLK, (blk + 1) * BLK)
                xt_t = xp.tile([128, 8, BLK], BF16, name="xt_t2", tag="xt")
                nc.sync.dma_start(xt_t[:], xt_v[:, :, ns])
                qt_sb = work.tile([128, 4, BLK], BF16, name="qt_sb", tag="qt")
                return xt_t, qt_sb

            def emit_qt_group(xt_t, qt_sb, jc):
                ps = pbig.tile([128, BLK], F32, name="ps_qt", tag="big")
                for dc in range(8):
                    nc.tensor.matmul(
                        ps[:],
                        wq_sb[:, dc, jc * 128 : (jc + 1) * 128],
                        xt_t[:, dc, :],
                        start=(dc == 0),
                        stop=(dc == 7),
                    )
                nc.scalar.activation(
                    qt_sb[:, jc, :], ps[:], AF.Identity,
                    bias=bq_sb[:, jc : jc + 1], scale=1.0,
                )

            def emit_pj(blk, o_sb, oc_range):
                ns = slice(blk * BLK, (blk + 1) * BLK)
                for oc in oc_range:
                    pj = pbig.tile([128, BLK], F32, name="ps_pj", tag="big")
                    for jc in range(4):
                        nc.tensor.matmul(
                            pj[:],
                            wp_sb[:, jc, oc * 128 : (oc + 1) * 128],
                            o_sb[:, jc, :],
                            start=(jc == 0),
                            stop=(jc == 3),
                        )
                    pj_sb = small.tile([128, BLK], F32, name="pj_sb", tag="pj", bufs=4)
                    if oc % 2 == 0:
                        nc.vector.tensor_copy(out=pj_sb[:], in_=pj[:])
                    else:
                        nc.scalar.copy(pj_sb[:], pj[:])
                    nc.sync.dma_start(out_v[:, oc, ns], pj_sb[:])

            cur = emit_qt_start(0)
            for jc in range(4):
                emit_qt_group(cur[0], cur[1], jc)

            prev_o = None  # (blk, o_sb) whose proj is still pending
            for blk in range(NBLK):
                qt_sb = cur[1]
                have_next = blk + 1 < NBLK
                if have_next:
                    nxt = emit_qt_start(blk + 1)

                    def filler(stage):
                        emit_qt_group(nxt[0], nxt[1], stage)
                else:
                    # last block: no next qt; fill PE with the pending proj
                    lo_blk, lo_sb = prev_o
                    prev_o = None

                    def filler(stage):
                        emit_pj(lo_blk, lo_sb, range(stage * 2, stage * 2 + 2))

                # q' (transposed [(h F), n]); q-side max cancels in the ratio
                qp_sb = work.tile([128, 4, BLK], BF16, name="qp_sb", tag="qp")
                for p in range(4):
                    ps = pbig.tile([128, BLK], F32, name="ps_qf", tag="big")
                    nc.tensor.matmul(
                        ps[:], pbdh_sb[:], qt_sb[:, p, :],
                        start=True, stop=True,
                    )
                    nc.scalar.activation(
                        qp_sb[:, p, :], ps[:], AF.Exp, bias=eps_sb[:], scale=1.0
                    )
                filler(0)

                # normalizer -> 1/norm (approx, 18 bits) -> bf16
                rns = []
                for p in range(4):
                    nrm = pnrm.tile([2, BLK], F32, name="nrm", tag="nrm")
                    nc.tensor.matmul(
                        nrm[:], ksbc[:, p, :], qp_sb[:, p, :],
                        start=True, stop=True,
                    )
                    rf = small.tile([2, BLK], F32, name="rf", tag="rf")
                    nc.vector.reciprocal_approx_fast(out=rf[:], in_=nrm[:])
                    rn = small.tile([2, BLK], BF16, name="rn", tag="rn")
                    if p % 2 == 0:
                        nc.scalar.copy(rn[:], rf[:])
                    else:
                        nc.vector.tensor_copy(out=rn[:], in_=rf[:])
                    rns.append(rn)
                filler(1)

                # broadcast 1/norm over each head's 64 partitions; divide q'
                q2s = []
                for p in range(4):
                    bc = pmid.tile([128, BLK], F32, name="ps_bc", tag="mid")
                    nc.tensor.matmul(
                        bc[:], oh2_sb[:], rns[p][:], start=True, stop=True
                    )
                    q2 = small.tile([128, BLK], BF16, name="q2", tag="q2", bufs=4)
                    nc.vector.tensor_mul(q2[:], qp_sb[:, p, :], bc[:])
                    q2s.append(q2)
                filler(2)

                o_sb = work.tile([128, 4, BLK], BF16, name="o_sb", tag="o", bufs=3)
                for p in range(4):
                    po = pmid.tile([128, BLK], F32, name="ps_o", tag="mid")
                    nc.tensor.matmul(
                        po[:], kvbd[:, p, :], q2s[p][:], start=True, stop=True
                    )
                    if p % 2 == 0:
                        nc.scalar.copy(o_sb[:, p, :], po[:])
                    else:
                        nc.vector.tensor_copy(out=o_sb[:, p, :], in_=po[:])
                filler(3)

                # flush the previous block's pending proj, keep ours pending
                if prev_o is not None:
                    emit_pj(prev_o[0], prev_o[1], range(8))
                prev_o = (blk, o_sb)

                if have_next:
                    cur = nxt

            # proj of the final block
            emit_pj(prev_o[0], prev_o[1], range(8))

            pmid.release()

    nc.compile()
    return nc


_NC = None


def _get_nc():
    global _NC
    if _NC is None:
        _NC = _build_nc()
    return _NC


def _host_inputs(x, W_qkv, b_qkv, W_proj, b_proj, proj_mat):
    x = np.asarray(x, dtype=np.float32)
    W_qkv = np.asarray(W_qkv, dtype=np.float32)
    b_qkv = np.asarray(b_qkv, dtype=np.float32)
    W_proj = np.asarray(W_proj, dtype=np.float32)
    proj_mat = np.asarray(proj_mat, dtype=np.float32)

    pt = (proj_mat.T * SCALE).astype(np.float32)  # [hd, F]
    pbd = np.zeros((128, 128), dtype=np.float32)
    pbd[:64, :64] = pt
    pbd[64:, 64:] = pt
    oh2 = np.zeros((2, 128), dtype=np.float32)
    oh2[0, :64] = 1.0
    oh2[1, 64:] = 1.0

    xts = [np.ascontiguousarray(x[b].T).astype(ml_dtypes.bfloat16) for b in range(4)]

    in_maps = []
    for c in range(8):
        b, g = c // 2, c % 2
        wqs = W_qkv[g * 512 : (g + 1) * 512]
        wks = W_qkv[D + g * 512 : D + (g + 1) * 512]
        wvs = W_qkv[2 * D + g * 512 : 2 * D + (g + 1) * 512]
        bqs = b_qkv[g * 512 : (g + 1) * 512]
        bks = b_qkv[D + g * 512 : D + (g + 1) * 512]
        bvs = b_qkv[2 * D + g * 512 : 2 * D + (g + 1) * 512]
        bvb = np.empty((128, 4, 64), dtype=np.float32)
        bv_r = bvs.reshape(4, 2, 64)
        for p in range(4):
            bvb[0:64, p, :] = bv_r[p, 0][None, :]
            bvb[64:128, p, :] = bv_r[p, 1][None, :]
        in_maps.append(
            {
                "xt": xts[b],
                "wq": np.ascontiguousarray(wqs.T).astype(ml_dtypes.bfloat16),
                "wk": np.ascontiguousarray(wks.T).astype(ml_dtypes.bfloat16),
                "wv": np.ascontiguousarray(wvs.T).astype(ml_dtypes.bfloat16),
                "wp": np.ascontiguousarray(
                    W_proj[:, g * 512 : (g + 1) * 512].T
                ).astype(ml_dtypes.bfloat16),
                "bq": np.ascontiguousarray(bqs.reshape(4, 128).T),
                "bk": np.ascontiguousarray(bks.reshape(4, 128).T),
                "bvb": bvb,
                "pbdh": pbd.astype(ml_dtypes.bfloat16),
                "oh2": oh2.astype(ml_dtypes.bfloat16),
            }
        )
    return in_maps


def kernel(x, W_qkv, b_qkv, W_proj, b_proj, proj_mat):
    b_proj = np.asarray(b_proj, dtype=np.float32)
    in_maps = _host_inputs(x, W_qkv, b_qkv, W_proj, b_proj, proj_mat)
    nc = _get_nc()
    res = run_bass_kernel_spmd(nc, in_maps, core_ids=list(range(8)))
    final = np.empty((4, N, D), dtype=np.float32)
    for b in range(4):
        acc = res.results[2 * b]["out"] + res.results[2 * b + 1]["out"]
        final[b] = acc.T + b_proj[None, :]
    return final


# revision 23
# speedup vs baseline: 1.0055x; 1.0055x over previous
"""FAVOR+ attention (Performer) Trainium2 kernel, 8-way sharded.

Sharding: 8 cores = 4 batches x 2 head-groups. Core c handles batch c//2 and
heads [8*(c%2), 8*(c%2)+8). The attention core (kv state) is fully local per
head; the output projection is computed as a per-core partial over its 512
input channels and the two partials per batch are summed on the host.

All matmuls run in bf16 (1 cycle/row on the PE); accumulation is fp32 in
PSUM. The output is bias-dominated, so bf16 operand rounding keeps the final
relative error at the few-1e-3 level.
"""

import numpy as np
import ml_dtypes

import concourse.bass as bass
import concourse.mybir as mybir
import concourse.tile as tile
from concourse import bacc
from concourse.bass_utils import run_bass_kernel_spmd

F32 = mybir.dt.float32
BF16 = mybir.dt.bfloat16
AF = mybir.ActivationFunctionType
ALU = mybir.AluOpType

N = 4096
D = 1024
HD = 64
NF = 64
EPS = 1e-4
BLK = 512  # n-block
NBLK = N // BLK
NCH = BLK // 128  # 128-row chunks per block
SCALE = float(HD) ** -0.25


def _build_nc():
    nc = bacc.Bacc("TRN2", target_bir_lowering=False, debug=False, num_devices=8)

    xt = nc.dram_tensor("xt", [D, N], BF16, kind="ExternalInput").ap()
    wqp = nc.dram_tensor("wqp", [D, 512], BF16, kind="ExternalInput").ap()
    wkp = nc.dram_tensor("wkp", [D, 512], BF16, kind="ExternalInput").ap()
    wv = nc.dram_tensor("wv", [D, 512], BF16, kind="ExternalInput").ap()
    wp = nc.dram_tensor("wp", [512, D], BF16, kind="ExternalInput").ap()
    bqpe = nc.dram_tensor("bqpe", [128, 4], F32, kind="ExternalInput").ap()
    bkp = nc.dram_tensor("bkp", [1, 512], BF16, kind="ExternalInput").ap()
    bvb = nc.dram_tensor("bvb", [128, 4, 64], F32, kind="ExternalInput").ap()
    oh2 = nc.dram_tensor("oh2", [2, 128], BF16, kind="ExternalInput").ap()
    out = nc.dram_tensor("out", [D, N], F32, kind="ExternalOutput").ap()

    xt_v = xt.rearrange("(dc p) n -> p dc n", p=128)  # [128, 8, 4096]
    wqp_v = wqp.rearrange("(dc p) j -> p dc j", p=128)  # [128, 8, 512]
    wkp_v = wkp.rearrange("(dc p) j -> p dc j", p=128)
    wv_v = wv.rearrange("(dc p) j -> p dc j", p=128)
    wp_v = wp.rearrange("(jc p) o -> p jc o", p=128)  # [128, 4, 1024]
    out_v = out.rearrange("(oc p) n -> p oc n", p=128)  # [128, 8, 4096]

    with tile.TileContext(nc) as tc:
        with (
            tc.tile_pool(name="consts", bufs=1) as consts,
            tc.tile_pool(name="xp", bufs=3) as xp,
            tc.tile_pool(name="work", bufs=2) as work,
            tc.tile_pool(name="small", bufs=4) as small,
            tc.tile_pool(name="pbig", bufs=4, space="PSUM") as pbig,
            tc.tile_pool(name="pnrm", bufs=2, space="PSUM") as pnrm,
        ):
            pkv = tc.alloc_tile_pool(name="pkv", bufs=1, space="PSUM")
            # ---- constants / weights (pass-A-critical loads first) ----
            wv_sb = consts.tile([128, 8, 512], BF16, name="wv_sb")
            nc.scalar.dma_start(wv_sb[:, 0:4, :], wv_v[:, 0:4, :])
            wkp_sb = consts.tile([128, 8, 512], BF16, name="wkp_sb")
            nc.scalar.dma_start(wkp_sb[:, 0:4, :], wkp_v[:, 0:4, :])
            bkp_sb = consts.tile([1, 512], BF16, name="bkp_sb")
            nc.scalar.dma_start(bkp_sb[:], bkp)
            nc.scalar.dma_start(wv_sb[:, 4:8, :], wv_v[:, 4:8, :])
            nc.scalar.dma_start(wkp_sb[:, 4:8, :], wkp_v[:, 4:8, :])
            eps_sb = consts.tile([128, 1], F32, name="eps_sb")
            nc.vector.memset(eps_sb[:], EPS)
            ones1_sb = consts.tile([1, 128], BF16, name="ones1_sb")
            nc.vector.memset(ones1_sb[:], 1.0)

            # declared now, loaded during pass A
            wqp_sb = consts.tile([128, 8, 512], BF16, name="wqp_sb")
            wp_sb = consts.tile([128, 4, 1024], BF16, name="wp_sb")
            oh2_sb = consts.tile([2, 128], BF16, name="oh2_sb")
            bqpe_sb = consts.tile([128, 4], F32, name="bqpe_sb")
            bvb_sb = consts.tile([128, 4, 64], F32, name="bvb_sb")

            # kv accumulators: pairs (0,1) in kvacc0, (2,3) in kvacc1.
            # Layout per pair: 129 cols (64 v-head0 | 64 v-head1 | ksum), stride 130.
            kvacc = [
                pkv.tile([128, 260], F32, name=f"kvacc{t}", tag=f"kvacc{t}")
                for t in range(2)
            ]

            # ================= pass A: k', v -> kv, ksum =================
            for blk in range(NBLK):
                ns = slice(blk * BLK, (blk + 1) * BLK)
                xt_t = xp.tile([128, 8, BLK], BF16, name="xt_t", tag="xt")
                if blk == 0:
                    nc.sync.dma_start(xt_t[:, 0:4, :], xt_v[:, 0:4, ns])
                    nc.sync.dma_start(xt_t[:, 4:8, :], xt_v[:, 4:8, ns])
                else:
                    nc.sync.dma_start(xt_t[:], xt_v[:, :, ns])

                # v and k-features (x @ Wkp + bkp, fused on host) per chunk
                v_sbs = []
                kp_sbs = []
                for c in range(NCH):
                    cs = slice(c * 128, (c + 1) * 128)
                    psv = pbig.tile([128, 512], F32, name="ps_v", tag="big")
                    for dc in range(8):
                        nc.tensor.matmul(
                            psv[:],
                            xt_t[:, dc, cs],
                            wv_sb[:, dc, :],
                            start=(dc == 0),
                            stop=(dc == 7),
                        )
                    v_sb = work.tile([128, 4, 132], BF16, name="v_sb", tag="v", bufs=5)
                    nc.scalar.copy(
                        v_sb[:, :, 0:128],
                        psv.rearrange("p (g j) -> p g j", j=128),
                    )
                    nc.vector.memset(v_sb[:, :, 128:129], 1.0)
                    v_sbs.append(v_sb)

                    psf = pbig.tile([128, 512], F32, name="ps_kf", tag="big")
                    for dc in range(8):
                        nc.tensor.matmul(
                            psf[:],
                            xt_t[:, dc, cs],
                            wkp_sb[:, dc, :],
                            start=(dc == 0),
                            stop=False,
                        )
                    nc.tensor.matmul(
                        psf[:], ones1_sb[:], bkp_sb[:], start=False, stop=True
                    )
                    psf_v = psf.rearrange("p (g f) -> p g f", f=64)  # [128, 8, 64]
                    mx = small.tile([128, 8], F32, name="mx", tag="mx")
                    nc.vector.reduce_max(mx[:], psf_v, axis=mybir.AxisListType.X)
                    karg = small.tile([128, 8, 64], F32, name="karg", tag="karg")
                    nc.vector.tensor_tensor(
                        karg[:], psf_v,
                        mx[:, :, None].to_broadcast([128, 8, 64]),
                        ALU.subtract,
                    )
                    kp_sb = work.tile([128, 4, 128], BF16, name="kp_sb", tag="kp", bufs=5)
                    nc.scalar.activation(
                        kp_sb.rearrange("p g (h f) -> p (g h) f", f=64),
                        karg[:], AF.Exp, bias=eps_sb[:], scale=1.0,
                    )
                    kp_sbs.append(kp_sb)

                # kv (+ksum) accumulation
                for c in range(NCH):
                    glob_first = blk == 0 and c == 0
                    glob_last = blk == NBLK - 1 and c == NCH - 1
                    for p in range(4):
                        base = (p % 2) * 130
                        nc.tensor.matmul(
                            kvacc[p // 2][:, base : base + 129],
                            kp_sbs[c][:, p, :],
                            v_sbs[c][:, p, 0:129],
                            start=(glob_first and p % 2 == 0),
                            stop=(glob_last and p % 2 == 1),
                        )

                if blk == 0:
                    # stream pass-B weights while pass A computes (gpsimd
                    # SWDGE queue; the sync queue keeps feeding xt blocks)
                    nc.gpsimd.dma_start(wqp_sb[:], wqp_v)
                    nc.gpsimd.dma_start(wp_sb[:], wp_v)
                    nc.gpsimd.dma_start(oh2_sb[:], oh2)
                    nc.gpsimd.dma_start(bqpe_sb[:], bqpe)
                    nc.gpsimd.dma_start(bvb_sb[:], bvb)

            # ============ assemble kv blockdiag + ksum columns ============
            kvbd = consts.tile([128, 4, 128], BF16, name="kvbd")
            ksbc = consts.tile([128, 4, 2], BF16, name="ksbc")
            nc.vector.memset(kvbd[:], 0.0)
            nc.vector.memset(ksbc[:], 0.0)
            for p in range(4):
                t = kvacc[p // 2]
                base = (p % 2) * 130
                ks = t[:, base + 128 : base + 129]
                nc.vector.tensor_copy(out=ksbc[0:64, p, 0:1], in_=ks[0:64])
                nc.vector.tensor_copy(out=ksbc[64:128, p, 1:2], in_=ks[64:128])
                # kv[h] += ksum[h] (x) bv[h], fold v-bias into kv
                nc.vector.scalar_tensor_tensor(
                    out=kvbd[0:64, p, 0:64],
                    in0=bvb_sb[0:64, p, :],
                    scalar=ks[0:64],
                    in1=t[0:64, base : base + 64],
                    op0=ALU.mult,
                    op1=ALU.add,
                )
                nc.vector.scalar_tensor_tensor(
                    out=kvbd[64:128, p, 64:128],
                    in0=bvb_sb[64:128, p, :],
                    scalar=ks[64:128],
                    in1=t[64:128, base + 64 : base + 128],
                    op0=ALU.mult,
                    op1=ALU.add,
                )

            # kv accumulator banks are dead now; hand them to pass B's
            # bc/po single-matmul stages so they don't churn the main ring.
            pkv.release()
            pmid = tc.alloc_tile_pool(name="pmid", bufs=2, space="PSUM")

            # ================= pass B: q', out, proj =================
            # q_proj^T comes straight from x @ Wqp (feature projection fused
            # into the weights on the host); exp bias carries bqp + eps.
            # The nrm -> bc -> po chain has an ACT/DVE hop between stages;
            # interleave each stage with one q-projection group of the NEXT
            # block so the PE never idles (keeps HAM at 2.4 GHz).
            def emit_qp_start(blk):
                ns = slice(blk * BLK, (blk + 1) * BLK)
                xt_t = xp.tile([128, 8, BLK], BF16, name="xt_t2", tag="xt")
                nc.sync.dma_start(xt_t[:], xt_v[:, :, ns])
                qp_sb = work.tile([128, 4, BLK], BF16, name="qp_sb", tag="qp")
                return xt_t, qp_sb

            def emit_qp_group(xt_t, qp_sb, p):
                ps = pbig.tile([128, BLK], F32, name="ps_qt", tag="big")
                for dc in range(8):
                    nc.tensor.matmul(
                        ps[:],
                        wqp_sb[:, dc, p * 128 : (p + 1) * 128],
                        xt_t[:, dc, :],
                        start=(dc == 0),
                        stop=(dc == 7),
                    )
                nc.scalar.activation(
                    qp_sb[:, p, :], ps[:], AF.Exp,
                    bias=bqpe_sb[:, p : p + 1], scale=1.0,
                )

            def emit_pj(blk, o_sb, oc_range):
                ns = slice(blk * BLK, (blk + 1) * BLK)
                for oc in oc_range:
                    pj = pbig.tile([128, BLK], F32, name="ps_pj", tag="big")
                    for jc in range(4):
                        nc.tensor.matmul(
                            pj[:],
                            wp_sb[:, jc, oc * 128 : (oc + 1) * 128],
                            o_sb[:, jc, :],
                            start=(jc == 0),
                            stop=(jc == 3),
                        )
                    pj_sb = small.tile([128, BLK], F32, name="pj_sb", tag="pj", bufs=4)
                    if oc % 2 == 0:
                        nc.vector.tensor_copy(out=pj_sb[:], in_=pj[:])
                    else:
                        nc.scalar.copy(pj_sb[:], pj[:])
                    nc.sync.dma_start(out_v[:, oc, ns], pj_sb[:])

            cur = emit_qp_start(0)
            for p in range(4):
                emit_qp_group(cur[0], cur[1], p)

            prev_o = None  # (blk, o_sb) whose proj is still pending
            for blk in range(NBLK):
                qp_sb = cur[1]
                have_next = blk + 1 < NBLK
                if have_next:
                    nxt = emit_qp_start(blk + 1)

                    def filler(stage):
                        emit_qp_group(nxt[0], nxt[1], stage)
                else:
                    # last block: no next q-projection; fill with pending proj
                    lo_blk, lo_sb = prev_o
                    prev_o = None

                    def filler(stage):
                        emit_pj(lo_blk, lo_sb, range(stage * 2, stage * 2 + 2))

                # normalizer -> 1/norm (approx, 18 bits) -> bf16
                rns = []
                for p in range(4):
                    nrm = pnrm.tile([2, BLK], F32, name="nrm", tag="nrm")
                    nc.tensor.matmul(
                        nrm[:], ksbc[:, p, :], qp_sb[:, p, :],
                        start=True, stop=True,
                    )
                    rf = small.tile([2, BLK], F32, name="rf", tag="rf")
                    nc.vector.reciprocal_approx_fast(out=rf[:], in_=nrm[:])
                    rn = small.tile([2, BLK], BF16, name="rn", tag="rn")
                    if p % 2 == 0:
                        nc.scalar.copy(rn[:], rf[:])
                    else:
                        nc.vector.tensor_copy(out=rn[:], in_=rf[:])
                    rns.append(rn)
                filler(0)

                # broadcast 1/norm over each head's 64 partitions; divide q'
                q2s = []
                for p in range(4):
                    bc = pmid.tile([128, BLK], F32, name="ps_bc", tag="mid")
                    nc.tensor.matmul(
                        bc[:], oh2_sb[:], rns[p][:], start=True, stop=True
                    )
                    q2 = small.tile([128, BLK], BF16, name="q2", tag="q2", bufs=4)
                    nc.vector.tensor_mul(q2[:], qp_sb[:, p, :], bc[:])
                    q2s.append(q2)
                filler(1)

                o_sb = work.tile([128, 4, BLK], BF16, name="o_sb", tag="o", bufs=3)
                for p in range(4):
                    po = pmid.tile([128, BLK], F32, name="ps_o", tag="mid")
                    nc.tensor.matmul(
                        po[:], kvbd[:, p, :], q2s[p][:], start=True, stop=True
                    )
                    if p % 2 == 0:
                        nc.scalar.copy(o_sb[:, p, :], po[:])
                    else:
                        nc.vector.tensor_copy(out=o_sb[:, p, :], in_=po[:])
                filler(2)
                filler(3)

                # flush the previous block's pending proj, keep ours pending
                if prev_o is not None:
                    emit_pj(prev_o[0], prev_o[1], range(8))
                prev_o = (blk, o_sb)

                if have_next:
                    cur = nxt

            # proj of the final block
            emit_pj(prev_o[0], prev_o[1], range(8))

            pmid.release()

    nc.compile()
    return nc


_NC = None


def _get_nc():
    global _NC
    if _NC is None:
        _NC = _build_nc()
    return _NC


def _host_inputs(x, W_qkv, b_qkv, W_proj, b_proj, proj_mat):
    x = np.asarray(x, dtype=np.float32)
    W_qkv = np.asarray(W_qkv, dtype=np.float32)
    b_qkv = np.asarray(b_qkv, dtype=np.float32)
    W_proj = np.asarray(W_proj, dtype=np.float32)
    proj_mat = np.asarray(proj_mat, dtype=np.float32)

    pt = (proj_mat.T * SCALE).astype(np.float32)  # [hd, F]
    oh2 = np.zeros((2, 128), dtype=np.float32)
    oh2[0, :64] = 1.0
    oh2[1, 64:] = 1.0

    xts = [np.ascontiguousarray(x[b].T).astype(ml_dtypes.bfloat16) for b in range(4)]

    def fuse(Wslc, bslc):
        # W_fused[:, (h f)] = sum_d W.T[:, (h d)] pt[d, f]; bias likewise
        wT = Wslc.T.reshape(D, 8, HD)
        wf = np.einsum("ahd,df->ahf", wT, pt).reshape(D, 512)
        bf = np.einsum("hd,df->hf", bslc.reshape(8, HD), pt).reshape(512)
        return wf, bf

    in_maps = []
    for c in range(8):
        b, g = c // 2, c % 2
        wqs = W_qkv[g * 512 : (g + 1) * 512]
        wks = W_qkv[D + g * 512 : D + (g + 1) * 512]
        wvs = W_qkv[2 * D + g * 512 : 2 * D + (g + 1) * 512]
        bqs = b_qkv[g * 512 : (g + 1) * 512]
        bks = b_qkv[D + g * 512 : D + (g + 1) * 512]
        bvs = b_qkv[2 * D + g * 512 : 2 * D + (g + 1) * 512]
        wqp, bqp = fuse(wqs, bqs)
        wkp, bkp = fuse(wks, bks)
        bvb = np.empty((128, 4, 64), dtype=np.float32)
        bv_r = bvs.reshape(4, 2, 64)
        for p in range(4):
            bvb[0:64, p, :] = bv_r[p, 0][None, :]
            bvb[64:128, p, :] = bv_r[p, 1][None, :]
        in_maps.append(
            {
                "xt": xts[b],
                "wqp": np.ascontiguousarray(wqp).astype(ml_dtypes.bfloat16),
                "wkp": np.ascontiguousarray(wkp).astype(ml_dtypes.bfloat16),
                "wv": np.ascontiguousarray(wvs.T).astype(ml_dtypes.bfloat16),
                "wp": np.ascontiguousarray(
                    W_proj[:, g * 512 : (g + 1) * 512].T
                ).astype(ml_dtypes.bfloat16),
                "bqpe": np.ascontiguousarray(
                    (bqp + EPS).reshape(4, 128).T
                ).astype(np.float32),
                "bkp": bkp.reshape(1, 512).astype(ml_dtypes.bfloat16),
                "bvb": bvb,
                "oh2": oh2.astype(ml_dtypes.bfloat16),
            }
        )
    return in_maps


def kernel(x, W_qkv, b_qkv, W_proj, b_proj, proj_mat):
    b_proj = np.asarray(b_proj, dtype=np.float32)
    in_maps = _host_inputs(x, W_qkv, b_qkv, W_proj, b_proj, proj_mat)
    nc = _get_nc()
    res = run_bass_kernel_spmd(nc, in_maps, core_ids=list(range(8)))
    final = np.empty((4, N, D), dtype=np.float32)
    for b in range(4):
        acc = res.results[2 * b]["out"] + res.results[2 * b + 1]["out"]
        final[b] = acc.T + b_proj[None, :]
    return final


# revision 25
# speedup vs baseline: 1.0372x; 1.0315x over previous
"""FAVOR+ attention (Performer) Trainium2 kernel, 8-way sharded.

Sharding: 8 cores = 4 batches x 2 head-groups. Core c handles batch c//2 and
heads [8*(c%2), 8*(c%2)+8). The attention core (kv state) is fully local per
head; the output projection is computed as a per-core partial over its 512
input channels and the two partials per batch are summed on the host.

All matmuls run in bf16 (1 cycle/row on the PE); accumulation is fp32 in
PSUM. The output is bias-dominated, so bf16 operand rounding keeps the final
relative error at the few-1e-3 level.
"""

import numpy as np
import ml_dtypes

import concourse.bass as bass
import concourse.mybir as mybir
import concourse.tile as tile
from concourse import bacc
from concourse.bass_utils import run_bass_kernel_spmd

F32 = mybir.dt.float32
BF16 = mybir.dt.bfloat16
AF = mybir.ActivationFunctionType
ALU = mybir.AluOpType

N = 4096
D = 1024
HD = 64
NF = 64
EPS = 1e-4
BLK = 512  # n-block
NBLK = N // BLK
NCH = BLK // 128  # 128-row chunks per block
SCALE = float(HD) ** -0.25


def _build_nc():
    nc = bacc.Bacc("TRN2", target_bir_lowering=False, debug=False, num_devices=8)

    xt = nc.dram_tensor("xt", [D, N], BF16, kind="ExternalInput").ap()
    wqp = nc.dram_tensor("wqp", [D, 512], BF16, kind="ExternalInput").ap()
    wkp = nc.dram_tensor("wkp", [D, 512], BF16, kind="ExternalInput").ap()
    wv = nc.dram_tensor("wv", [D, 512], BF16, kind="ExternalInput").ap()
    wp = nc.dram_tensor("wp", [512, D], BF16, kind="ExternalInput").ap()
    bqpe = nc.dram_tensor("bqpe", [128, 4], F32, kind="ExternalInput").ap()
    bkp = nc.dram_tensor("bkp", [1, 512], BF16, kind="ExternalInput").ap()
    bvb = nc.dram_tensor("bvb", [128, 4, 64], F32, kind="ExternalInput").ap()
    oh2 = nc.dram_tensor("oh2", [2, 128], BF16, kind="ExternalInput").ap()
    out = nc.dram_tensor("out", [D, N], F32, kind="ExternalOutput").ap()

    xt_v = xt.rearrange("(dc p) n -> p dc n", p=128)  # [128, 8, 4096]
    wqp_v = wqp.rearrange("(dc p) j -> p dc j", p=128)  # [128, 8, 512]
    wkp_v = wkp.rearrange("(dc p) j -> p dc j", p=128)
    wv_v = wv.rearrange("(dc p) j -> p dc j", p=128)
    wp_v = wp.rearrange("(jc p) o -> p jc o", p=128)  # [128, 4, 1024]
    out_v = out.rearrange("(oc p) n -> p oc n", p=128)  # [128, 8, 4096]

    with tile.TileContext(nc) as tc:
        with (
            tc.tile_pool(name="consts", bufs=1) as consts,
            tc.tile_pool(name="xp", bufs=3) as xp,
            tc.tile_pool(name="work", bufs=2) as work,
            tc.tile_pool(name="small", bufs=4) as small,
            tc.tile_pool(name="pbig", bufs=4, space="PSUM") as pbig,
            tc.tile_pool(name="pnrm", bufs=2, space="PSUM") as pnrm,
        ):
            pkv = tc.alloc_tile_pool(name="pkv", bufs=1, space="PSUM")
            # ---- constants / weights (pass-A-critical loads first) ----
            wv_sb = consts.tile([128, 8, 512], BF16, name="wv_sb")
            nc.scalar.dma_start(wv_sb[:, 0:4, :], wv_v[:, 0:4, :])
            wkp_sb = consts.tile([128, 8, 512], BF16, name="wkp_sb")
            nc.gpsimd.dma_start(wkp_sb[:, 0:4, :], wkp_v[:, 0:4, :])
            bkp_sb = consts.tile([1, 512], BF16, name="bkp_sb")
            nc.gpsimd.dma_start(bkp_sb[:], bkp)
            nc.scalar.dma_start(wv_sb[:, 4:8, :], wv_v[:, 4:8, :])
            nc.gpsimd.dma_start(wkp_sb[:, 4:8, :], wkp_v[:, 4:8, :])
            eps_sb = consts.tile([128, 1], F32, name="eps_sb")
            nc.vector.memset(eps_sb[:], EPS)
            ones1_sb = consts.tile([1, 128], BF16, name="ones1_sb")
            nc.vector.memset(ones1_sb[:], 1.0)

            # declared now, loaded during pass A
            wqp_sb = consts.tile([128, 8, 512], BF16, name="wqp_sb")
            wp_sb = consts.tile([128, 4, 1024], BF16, name="wp_sb")
            oh2_sb = consts.tile([2, 128], BF16, name="oh2_sb")
            bqpe_sb = consts.tile([128, 4], F32, name="bqpe_sb")
            bvb_sb = consts.tile([128, 4, 64], F32, name="bvb_sb")

            # kv accumulators: pairs (0,1) in kvacc0, (2,3) in kvacc1.
            # Layout per pair: 129 cols (64 v-head0 | 64 v-head1 | ksum), stride 130.
            kvacc = [
                pkv.tile([128, 260], F32, name=f"kvacc{t}", tag=f"kvacc{t}")
                for t in range(2)
            ]

            # ================= pass A: k', v -> kv, ksum =================
            for blk in range(NBLK):
                ns = slice(blk * BLK, (blk + 1) * BLK)
                xt_t = xp.tile([128, 8, BLK], BF16, name="xt_t", tag="xt")
                if blk == 0:
                    nc.sync.dma_start(xt_t[:, 0:4, :], xt_v[:, 0:4, ns])
                    nc.sync.dma_start(xt_t[:, 4:8, :], xt_v[:, 4:8, ns])
                else:
                    nc.sync.dma_start(xt_t[:], xt_v[:, :, ns])

                # v and k-features (x @ Wkp + bkp, fused on host) per chunk
                v_sbs = []
                kp_sbs = []
                for c in range(NCH):
                    cs = slice(c * 128, (c + 1) * 128)
                    psv = pbig.tile([128, 512], F32, name="ps_v", tag="big")
                    for dc in range(8):
                        nc.tensor.matmul(
                            psv[:],
                            xt_t[:, dc, cs],
                            wv_sb[:, dc, :],
                            start=(dc == 0),
                            stop=(dc == 7),
                        )
                    v_sb = work.tile([128, 4, 132], BF16, name="v_sb", tag="v", bufs=5)
                    nc.scalar.copy(
                        v_sb[:, :, 0:128],
                        psv.rearrange("p (g j) -> p g j", j=128),
                    )
                    nc.vector.memset(v_sb[:, :, 128:129], 1.0)
                    v_sbs.append(v_sb)

                    psf = pbig.tile([128, 512], F32, name="ps_kf", tag="big")
                    for dc in range(8):
                        nc.tensor.matmul(
                            psf[:],
                            xt_t[:, dc, cs],
                            wkp_sb[:, dc, :],
                            start=(dc == 0),
                            stop=False,
                        )
                    nc.tensor.matmul(
                        psf[:], ones1_sb[:], bkp_sb[:], start=False, stop=True
                    )
                    psf_v = psf.rearrange("p (g f) -> p g f", f=64)  # [128, 8, 64]
                    mx = small.tile([128, 8], F32, name="mx", tag="mx")
                    nc.vector.reduce_max(mx[:], psf_v, axis=mybir.AxisListType.X)
                    karg = small.tile([128, 8, 64], F32, name="karg", tag="karg")
                    nc.vector.tensor_tensor(
                        karg[:], psf_v,
                        mx[:, :, None].to_broadcast([128, 8, 64]),
                        ALU.subtract,
                    )
                    kp_sb = work.tile([128, 4, 128], BF16, name="kp_sb", tag="kp", bufs=5)
                    nc.scalar.activation(
                        kp_sb.rearrange("p g (h f) -> p (g h) f", f=64),
                        karg[:], AF.Exp, bias=eps_sb[:], scale=1.0,
                    )
                    kp_sbs.append(kp_sb)

                # kv (+ksum) accumulation
                for c in range(NCH):
                    glob_first = blk == 0 and c == 0
                    glob_last = blk == NBLK - 1 and c == NCH - 1
                    for p in range(4):
                        base = (p % 2) * 130
                        nc.tensor.matmul(
                            kvacc[p // 2][:, base : base + 129],
                            kp_sbs[c][:, p, :],
                            v_sbs[c][:, p, 0:129],
                            start=(glob_first and p % 2 == 0),
                            stop=(glob_last and p % 2 == 1),
                        )

                if blk == 0:
                    # stream pass-B weights while pass A computes (gpsimd
                    # SWDGE queue; the sync queue keeps feeding xt blocks)
                    nc.gpsimd.dma_start(wqp_sb[:], wqp_v)
                    nc.gpsimd.dma_start(wp_sb[:], wp_v)
                    nc.gpsimd.dma_start(oh2_sb[:], oh2)
                    nc.gpsimd.dma_start(bqpe_sb[:], bqpe)
                    nc.gpsimd.dma_start(bvb_sb[:], bvb)

            # ============ assemble kv blockdiag + ksum columns ============
            kvbd = consts.tile([128, 4, 128], BF16, name="kvbd")
            ksbc = consts.tile([128, 4, 2], BF16, name="ksbc")
            nc.vector.memset(kvbd[:], 0.0)
            nc.vector.memset(ksbc[:], 0.0)
            for p in range(4):
                t = kvacc[p // 2]
                base = (p % 2) * 130
                ks = t[:, base + 128 : base + 129]
                nc.vector.tensor_copy(out=ksbc[0:64, p, 0:1], in_=ks[0:64])
                nc.vector.tensor_copy(out=ksbc[64:128, p, 1:2], in_=ks[64:128])
                # kv[h] += ksum[h] (x) bv[h], fold v-bias into kv
                nc.vector.scalar_tensor_tensor(
                    out=kvbd[0:64, p, 0:64],
                    in0=bvb_sb[0:64, p, :],
                    scalar=ks[0:64],
                    in1=t[0:64, base : base + 64],
                    op0=ALU.mult,
                    op1=ALU.add,
                )
                nc.vector.scalar_tensor_tensor(
                    out=kvbd[64:128, p, 64:128],
                    in0=bvb_sb[64:128, p, :],
                    scalar=ks[64:128],
                    in1=t[64:128, base + 64 : base + 128],
                    op0=ALU.mult,
                    op1=ALU.add,
                )

            # kv accumulator banks are dead now; hand them to pass B's
            # bc/po single-matmul stages so they don't churn the main ring.
            pkv.release()
            pmid = tc.alloc_tile_pool(name="pmid", bufs=2, space="PSUM")

            # ================= pass B: q', out, proj =================
            # q_proj^T comes straight from x @ Wqp (feature projection fused
            # into the weights on the host); exp bias carries bqp + eps.
            # The nrm -> bc -> po chain has an ACT/DVE hop between stages;
            # interleave each stage with one q-projection group of the NEXT
            # block so the PE never idles (keeps HAM at 2.4 GHz).
            def emit_qp_start(blk):
                ns = slice(blk * BLK, (blk + 1) * BLK)
                xt_t = xp.tile([128, 8, BLK], BF16, name="xt_t2", tag="xt")
                nc.sync.dma_start(xt_t[:], xt_v[:, :, ns])
                qp_sb = work.tile([128, 4, BLK], BF16, name="qp_sb", tag="qp")
                return xt_t, qp_sb

            def emit_qp_group(xt_t, qp_sb, p):
                ps = pbig.tile([128, BLK], F32, name="ps_qt", tag="big")
                for dc in range(8):
                    nc.tensor.matmul(
                        ps[:],
                        wqp_sb[:, dc, p * 128 : (p + 1) * 128],
                        xt_t[:, dc, :],
                        start=(dc == 0),
                        stop=(dc == 7),
                    )
                nc.scalar.activation(
                    qp_sb[:, p, :], ps[:], AF.Exp,
                    bias=bqpe_sb[:, p : p + 1], scale=1.0,
                )

            def emit_pj(blk, o_sb, oc_range):
                ns = slice(blk * BLK, (blk + 1) * BLK)
                for oc in oc_range:
                    pj = pbig.tile([128, BLK], F32, name="ps_pj", tag="big")
                    for jc in range(4):
                        nc.tensor.matmul(
                            pj[:],
                            wp_sb[:, jc, oc * 128 : (oc + 1) * 128],
                            o_sb[:, jc, :],
                            start=(jc == 0),
                            stop=(jc == 3),
                        )
                    pj_sb = small.tile([128, BLK], F32, name="pj_sb", tag="pj", bufs=4)
                    if oc % 2 == 0:
                        nc.vector.tensor_copy(out=pj_sb[:], in_=pj[:])
                    else:
                        nc.scalar.copy(pj_sb[:], pj[:])
                    nc.sync.dma_start(out_v[:, oc, ns], pj_sb[:])

            cur = emit_qp_start(0)
            for p in range(4):
                emit_qp_group(cur[0], cur[1], p)

            prev_o = None  # (blk, o_sb) whose proj is still pending
            for blk in range(NBLK):
                qp_sb = cur[1]
                have_next = blk + 1 < NBLK
                if have_next:
                    nxt = emit_qp_start(blk + 1)

                    def filler(stage):
                        emit_qp_group(nxt[0], nxt[1], stage)
                else:
                    # last block: no next q-projection; fill with pending proj
                    lo_blk, lo_sb = prev_o
                    prev_o = None

                    def filler(stage):
                        emit_pj(lo_blk, lo_sb, range(stage * 2, stage * 2 + 2))

                # normalizer -> 1/norm (approx, 18 bits) -> bf16
                rns = []
                for p in range(4):
                    nrm = pnrm.tile([2, BLK], F32, name="nrm", tag="nrm")
                    nc.tensor.matmul(
                        nrm[:], ksbc[:, p, :], qp_sb[:, p, :],
                        start=True, stop=True,
                    )
                    rf = small.tile([2, BLK], F32, name="rf", tag="rf")
                    nc.vector.reciprocal_approx_fast(out=rf[:], in_=nrm[:])
                    rn = small.tile([2, BLK], BF16, name="rn", tag="rn")
                    if p % 2 == 0:
                        nc.scalar.copy(rn[:], rf[:])
                    else:
                        nc.vector.tensor_copy(out=rn[:], in_=rf[:])
                    rns.append(rn)
                filler(0)

                # broadcast 1/norm over each head's 64 partitions; divide q'
                q2s = []
                for p in range(4):
                    bc = pmid.tile([128, BLK], F32, name="ps_bc", tag="mid")
                    nc.tensor.matmul(
                        bc[:], oh2_sb[:], rns[p][:], start=True, stop=True
                    )
                    q2 = small.tile([128, BLK], BF16, name="q2", tag="q2", bufs=4)
                    nc.vector.tensor_mul(q2[:], qp_sb[:, p, :], bc[:])
                    q2s.append(q2)
                filler(1)

                o_sb = work.tile([128, 4, BLK], BF16, name="o_sb", tag="o", bufs=3)
                for p in range(4):
                    po = pmid.tile([128, BLK], F32, name="ps_o", tag="mid")
                    nc.tensor.matmul(
                        po[:], kvbd[:, p, :], q2s[p][:], start=True, stop=True
                    )
                    if p % 2 == 0:
                        nc.scalar.copy(o_sb[:, p, :], po[:])
                    else:
                        nc.vector.tensor_copy(out=o_sb[:, p, :], in_=po[:])
                filler(2)
                filler(3)

                # flush the previous block's pending proj, keep ours pending
                if prev_o is not None:
                    emit_pj(prev_o[0], prev_o[1], range(8))
                prev_o = (blk, o_sb)

                if have_next:
                    cur = nxt

            # proj of the final block
            emit_pj(prev_o[0], prev_o[1], range(8))

            pmid.release()

    nc.compile()
    return nc


_NC = None


def _get_nc():
    global _NC
    if _NC is None:
        _NC = _build_nc()
    return _NC


def _host_inputs(x, W_qkv, b_qkv, W_proj, b_proj, proj_mat):
    x = np.asarray(x, dtype=np.float32)
    W_qkv = np.asarray(W_qkv, dtype=np.float32)
    b_qkv = np.asarray(b_qkv, dtype=np.float32)
    W_proj = np.asarray(W_proj, dtype=np.float32)
    proj_mat = np.asarray(proj_mat, dtype=np.float32)

    pt = (proj_mat.T * SCALE).astype(np.float32)  # [hd, F]
    oh2 = np.zeros((2, 128), dtype=np.float32)
    oh2[0, :64] = 1.0
    oh2[1, 64:] = 1.0

    xts = [np.ascontiguousarray(x[b].T).astype(ml_dtypes.bfloat16) for b in range(4)]

    def fuse(Wslc, bslc):
        # W_fused[:, (h f)] = sum_d W.T[:, (h d)] pt[d, f]; bias likewise
        wT = Wslc.T.reshape(D, 8, HD)
        wf = np.einsum("ahd,df->ahf", wT, pt).reshape(D, 512)
        bf = np.einsum("hd,df->hf", bslc.reshape(8, HD), pt).reshape(512)
        return wf, bf

    in_maps = []
    for c in range(8):
        b, g = c // 2, c % 2
        wqs = W_qkv[g * 512 : (g + 1) * 512]
        wks = W_qkv[D + g * 512 : D + (g + 1) * 512]
        wvs = W_qkv[2 * D + g * 512 : 2 * D + (g + 1) * 512]
        bqs = b_qkv[g * 512 : (g + 1) * 512]
        bks = b_qkv[D + g * 512 : D + (g + 1) * 512]
        bvs = b_qkv[2 * D + g * 512 : 2 * D + (g + 1) * 512]
        wqp, bqp = fuse(wqs, bqs)
        wkp, bkp = fuse(wks, bks)
        bvb = np.empty((128, 4, 64), dtype=np.float32)
        bv_r = bvs.reshape(4, 2, 64)
        for p in range(4):
            bvb[0:64, p, :] = bv_r[p, 0][None, :]
            bvb[64:128, p, :] = bv_r[p, 1][None, :]
        in_maps.append(
            {
                "xt": xts[b],
                "wqp": np.ascontiguousarray(wqp).astype(ml_dtypes.bfloat16),
                "wkp": np.ascontiguousarray(wkp).astype(ml_dtypes.bfloat16),
                "wv": np.ascontiguousarray(wvs.T).astype(ml_dtypes.bfloat16),
                "wp": np.ascontiguousarray(
                    W_proj[:, g * 512 : (g + 1) * 512].T
                ).astype(ml_dtypes.bfloat16),
                "bqpe": np.ascontiguousarray(
                    (bqp + EPS).reshape(4, 128).T
                ).astype(np.float32),
                "bkp": bkp.reshape(1, 512).astype(ml_dtypes.bfloat16),
                "bvb": bvb,
                "oh2": oh2.astype(ml_dtypes.bfloat16),
            }
        )
    return in_maps


def kernel(x, W_qkv, b_qkv, W_proj, b_proj, proj_mat):
    b_proj = np.asarray(b_proj, dtype=np.float32)
    in_maps = _host_inputs(x, W_qkv, b_qkv, W_proj, b_proj, proj_mat)
    nc = _get_nc()
    res = run_bass_kernel_spmd(nc, in_maps, core_ids=list(range(8)))
    final = np.empty((4, N, D), dtype=np.float32)
    for b in range(4):
        acc = res.results[2 * b]["out"] + res.results[2 * b + 1]["out"]
        final[b] = acc.T + b_proj[None, :]
    return final


# revision 26
# speedup vs baseline: 1.0444x; 1.0070x over previous
"""FAVOR+ attention (Performer) Trainium2 kernel, 8-way sharded.

Sharding: 8 cores = 4 batches x 2 head-groups. Core c handles batch c//2 and
heads [8*(c%2), 8*(c%2)+8). The attention core (kv state) is fully local per
head; the output projection is computed as a per-core partial over its 512
input channels and the two partials per batch are summed on the host.

All matmuls run in bf16 (1 cycle/row on the PE); accumulation is fp32 in
PSUM. The output is bias-dominated, so bf16 operand rounding keeps the final
relative error at the few-1e-3 level.
"""

import numpy as np
import ml_dtypes

import concourse.mybir as mybir
import concourse.tile as tile
from concourse import bacc
from concourse.bass_utils import run_bass_kernel_spmd

F32 = mybir.dt.float32
BF16 = mybir.dt.bfloat16
AF = mybir.ActivationFunctionType
ALU = mybir.AluOpType

N = 4096
D = 1024
HD = 64
NF = 64
EPS = 1e-4
BLK = 512  # n-block
NBLK = N // BLK
NCH = BLK // 128  # 128-row chunks per block
SCALE = float(HD) ** -0.25


def _build_nc():
    nc = bacc.Bacc("TRN2", target_bir_lowering=False, debug=False, num_devices=8)

    xt = nc.dram_tensor("xt", [D, N], BF16, kind="ExternalInput").ap()
    wqp = nc.dram_tensor("wqp", [D, 512], BF16, kind="ExternalInput").ap()
    wkp = nc.dram_tensor("wkp", [D, 512], BF16, kind="ExternalInput").ap()
    wv = nc.dram_tensor("wv", [D, 512], BF16, kind="ExternalInput").ap()
    wp = nc.dram_tensor("wp", [512, D], BF16, kind="ExternalInput").ap()
    bqpe = nc.dram_tensor("bqpe", [128, 4], F32, kind="ExternalInput").ap()
    bkp = nc.dram_tensor("bkp", [1, 512], BF16, kind="ExternalInput").ap()
    bvb = nc.dram_tensor("bvb", [128, 4, 64], F32, kind="ExternalInput").ap()
    oh2 = nc.dram_tensor("oh2", [2, 128], BF16, kind="ExternalInput").ap()
    out = nc.dram_tensor("out", [D, N], F32, kind="ExternalOutput").ap()

    xt_v = xt.rearrange("(dc p) n -> p dc n", p=128)  # [128, 8, 4096]
    wqp_v = wqp.rearrange("(dc p) j -> p dc j", p=128)  # [128, 8, 512]
    wkp_v = wkp.rearrange("(dc p) j -> p dc j", p=128)
    wv_v = wv.rearrange("(dc p) j -> p dc j", p=128)
    wp_v = wp.rearrange("(jc p) o -> p jc o", p=128)  # [128, 4, 1024]
    out_v = out.rearrange("(oc p) n -> p oc n", p=128)  # [128, 8, 4096]

    with tile.TileContext(nc) as tc:
        with (
            tc.tile_pool(name="consts", bufs=1) as consts,
            tc.tile_pool(name="xp", bufs=4) as xp,
            tc.tile_pool(name="work", bufs=2) as work,
            tc.tile_pool(name="small", bufs=4) as small,
            tc.tile_pool(name="pbig", bufs=4, space="PSUM") as pbig,
            tc.tile_pool(name="pnrm", bufs=2, space="PSUM") as pnrm,
        ):
            pkv = tc.alloc_tile_pool(name="pkv", bufs=1, space="PSUM")
            # ---- constants / weights (pass-A-critical loads first) ----
            wv_sb = consts.tile([128, 8, 512], BF16, name="wv_sb")
            nc.scalar.dma_start(wv_sb[:, 0:4, :], wv_v[:, 0:4, :])
            wkp_sb = consts.tile([128, 8, 512], BF16, name="wkp_sb")
            nc.gpsimd.dma_start(wkp_sb[:, 0:4, :], wkp_v[:, 0:4, :])
            bkp_sb = consts.tile([1, 512], BF16, name="bkp_sb")
            nc.gpsimd.dma_start(bkp_sb[:], bkp)
            nc.scalar.dma_start(wv_sb[:, 4:8, :], wv_v[:, 4:8, :])
            nc.gpsimd.dma_start(wkp_sb[:, 4:8, :], wkp_v[:, 4:8, :])
            eps_sb = consts.tile([128, 1], F32, name="eps_sb")
            nc.vector.memset(eps_sb[:], EPS)
            ones1_sb = consts.tile([1, 128], BF16, name="ones1_sb")
            nc.vector.memset(ones1_sb[:], 1.0)

            # declared now, loaded during pass A
            wqp_sb = consts.tile([128, 8, 512], BF16, name="wqp_sb")
            wp_sb = consts.tile([128, 4, 1024], BF16, name="wp_sb")
            oh2_sb = consts.tile([2, 128], BF16, name="oh2_sb")
            bqpe_sb = consts.tile([128, 4], F32, name="bqpe_sb")
            bvb_sb = consts.tile([128, 4, 64], F32, name="bvb_sb")

            # kv accumulators: pairs (0,1) in kvacc0, (2,3) in kvacc1.
            # Layout per pair: 129 cols (64 v-head0 | 64 v-head1 | ksum), stride 130.
            kvacc = [
                pkv.tile([128, 260], F32, name=f"kvacc{t}", tag=f"kvacc{t}")
                for t in range(2)
            ]

            # ================= pass A: k', v -> kv, ksum =================
            for blk in range(NBLK):
                ns = slice(blk * BLK, (blk + 1) * BLK)
                xt_t = xp.tile([128, 8, BLK], BF16, name="xt_t", tag="xt")
                if blk == 0:
                    nc.sync.dma_start(xt_t[:, 0:4, :], xt_v[:, 0:4, ns])
                    nc.sync.dma_start(xt_t[:, 4:8, :], xt_v[:, 4:8, ns])
                else:
                    nc.sync.dma_start(xt_t[:], xt_v[:, :, ns])

                # v and k-features (x @ Wkp + bkp, fused on host) per chunk
                v_sbs = []
                kp_sbs = []
                for c in range(NCH):
                    cs = slice(c * 128, (c + 1) * 128)
                    psv = pbig.tile([128, 512], F32, name="ps_v", tag="big")
                    for dc in range(8):
                        nc.tensor.matmul(
                            psv[:],
                            xt_t[:, dc, cs],
                            wv_sb[:, dc, :],
                            start=(dc == 0),
                            stop=(dc == 7),
                        )
                    v_sb = work.tile([128, 4, 132], BF16, name="v_sb", tag="v", bufs=5)
                    nc.scalar.copy(
                        v_sb[:, :, 0:128],
                        psv.rearrange("p (g j) -> p g j", j=128),
                    )
                    nc.vector.memset(v_sb[:, :, 128:129], 1.0)
                    v_sbs.append(v_sb)

                    psf = pbig.tile([128, 512], F32, name="ps_kf", tag="big")
                    for dc in range(8):
                        nc.tensor.matmul(
                            psf[:],
                            xt_t[:, dc, cs],
                            wkp_sb[:, dc, :],
                            start=(dc == 0),
                            stop=False,
                        )
                    nc.tensor.matmul(
                        psf[:], ones1_sb[:], bkp_sb[:], start=False, stop=True
                    )
                    psf_v = psf.rearrange("p (g f) -> p g f", f=64)  # [128, 8, 64]
                    mx = small.tile([128, 8], F32, name="mx", tag="mx")
                    nc.vector.reduce_max(mx[:], psf_v, axis=mybir.AxisListType.X)
                    karg = small.tile([128, 8, 64], F32, name="karg", tag="karg")
                    nc.vector.tensor_tensor(
                        karg[:], psf_v,
                        mx[:, :, None].to_broadcast([128, 8, 64]),
                        ALU.subtract,
                    )
                    kp_sb = work.tile([128, 4, 128], BF16, name="kp_sb", tag="kp", bufs=5)
                    nc.scalar.activation(
                        kp_sb.rearrange("p g (h f) -> p (g h) f", f=64),
                        karg[:], AF.Exp, bias=eps_sb[:], scale=1.0,
                    )
                    kp_sbs.append(kp_sb)

                # kv (+ksum) accumulation
                for c in range(NCH):
                    glob_first = blk == 0 and c == 0
                    glob_last = blk == NBLK - 1 and c == NCH - 1
                    for p in range(4):
                        base = (p % 2) * 130
                        nc.tensor.matmul(
                            kvacc[p // 2][:, base : base + 129],
                            kp_sbs[c][:, p, :],
                            v_sbs[c][:, p, 0:129],
                            start=(glob_first and p % 2 == 0),
                            stop=(glob_last and p % 2 == 1),
                        )

                if blk == 0:
                    # stream pass-B weights while pass A computes (gpsimd
                    # SWDGE queue; the sync queue keeps feeding xt blocks)
                    nc.gpsimd.dma_start(wqp_sb[:], wqp_v)
                    nc.gpsimd.dma_start(wp_sb[:], wp_v)
                    nc.gpsimd.dma_start(oh2_sb[:], oh2)
                    nc.gpsimd.dma_start(bqpe_sb[:], bqpe)
                    nc.gpsimd.dma_start(bvb_sb[:], bvb)

            # ============ assemble kv blockdiag + ksum columns ============
            kvbd = consts.tile([128, 4, 128], BF16, name="kvbd")
            ksbc = consts.tile([128, 4, 2], BF16, name="ksbc")
            nc.vector.memset(kvbd[:], 0.0)
            nc.vector.memset(ksbc[:], 0.0)
            for p in range(4):
                t = kvacc[p // 2]
                base = (p % 2) * 130
                ks = t[:, base + 128 : base + 129]
                nc.vector.tensor_copy(out=ksbc[0:64, p, 0:1], in_=ks[0:64])
                nc.vector.tensor_copy(out=ksbc[64:128, p, 1:2], in_=ks[64:128])
                # kv[h] += ksum[h] (x) bv[h], fold v-bias into kv
                nc.vector.scalar_tensor_tensor(
                    out=kvbd[0:64, p, 0:64],
                    in0=bvb_sb[0:64, p, :],
                    scalar=ks[0:64],
                    in1=t[0:64, base : base + 64],
                    op0=ALU.mult,
                    op1=ALU.add,
                )
                nc.vector.scalar_tensor_tensor(
                    out=kvbd[64:128, p, 64:128],
                    in0=bvb_sb[64:128, p, :],
                    scalar=ks[64:128],
                    in1=t[64:128, base + 64 : base + 128],
                    op0=ALU.mult,
                    op1=ALU.add,
                )

            # kv accumulator banks are dead now; hand them to pass B's
            # bc/po single-matmul stages so they don't churn the main ring.
            pkv.release()
            pmid = tc.alloc_tile_pool(name="pmid", bufs=2, space="PSUM")

            # ================= pass B: q', out, proj =================
            # q_proj^T comes straight from x @ Wqp (feature projection fused
            # into the weights on the host); exp bias carries bqp + eps.
            # The nrm -> bc -> po chain has an ACT/DVE hop between stages;
            # interleave each stage with one q-projection group of the NEXT
            # block so the PE never idles (keeps HAM at 2.4 GHz).
            def emit_qp_start(blk):
                ns = slice(blk * BLK, (blk + 1) * BLK)
                xt_t = xp.tile([128, 8, BLK], BF16, name="xt_t2", tag="xt")
                nc.sync.dma_start(xt_t[:], xt_v[:, :, ns])
                qp_sb = work.tile([128, 4, BLK], BF16, name="qp_sb", tag="qp")
                return xt_t, qp_sb

            def emit_qp_group(xt_t, qp_sb, p):
                ps = pbig.tile([128, BLK], F32, name="ps_qt", tag="big")
                for dc in range(8):
                    nc.tensor.matmul(
                        ps[:],
                        wqp_sb[:, dc, p * 128 : (p + 1) * 128],
                        xt_t[:, dc, :],
                        start=(dc == 0),
                        stop=(dc == 7),
                    )
                nc.scalar.activation(
                    qp_sb[:, p, :], ps[:], AF.Exp,
                    bias=bqpe_sb[:, p : p + 1], scale=1.0,
                )

            def emit_pj(blk, o_sb, oc_range):
                ns = slice(blk * BLK, (blk + 1) * BLK)
                for oc in oc_range:
                    pj = pbig.tile([128, BLK], F32, name="ps_pj", tag="big")
                    for jc in range(4):
                        nc.tensor.matmul(
                            pj[:],
                            wp_sb[:, jc, oc * 128 : (oc + 1) * 128],
                            o_sb[:, jc, :],
                            start=(jc == 0),
                            stop=(jc == 3),
                        )
                    pj_sb = small.tile([128, BLK], F32, name="pj_sb", tag="pj", bufs=6)
                    if oc % 2 == 0:
                        nc.vector.tensor_copy(out=pj_sb[:], in_=pj[:])
                    else:
                        nc.scalar.copy(pj_sb[:], pj[:])
                    nc.sync.dma_start(out_v[:, oc, ns], pj_sb[:])

            cur = emit_qp_start(0)
            for p in range(4):
                emit_qp_group(cur[0], cur[1], p)

            prev_o = None  # (blk, o_sb) whose proj is still pending
            for blk in range(NBLK):
                qp_sb = cur[1]
                have_next = blk + 1 < NBLK
                if have_next:
                    nxt = emit_qp_start(blk + 1)

                    def filler(stage):
                        emit_qp_group(nxt[0], nxt[1], stage)
                else:
                    # last block: no next q-projection; fill with pending proj
                    lo_blk, lo_sb = prev_o
                    prev_o = None

                    def filler(stage):
                        emit_pj(lo_blk, lo_sb, range(stage * 2, stage * 2 + 2))

                # normalizer -> 1/norm (approx, 18 bits) -> bf16
                rns = []
                for p in range(4):
                    nrm = pnrm.tile([2, BLK], F32, name="nrm", tag="nrm")
                    nc.tensor.matmul(
                        nrm[:], ksbc[:, p, :], qp_sb[:, p, :],
                        start=True, stop=True,
                    )
                    rf = small.tile([2, BLK], F32, name="rf", tag="rf")
                    nc.vector.reciprocal_approx_fast(out=rf[:], in_=nrm[:])
                    rn = small.tile([2, BLK], BF16, name="rn", tag="rn")
                    if p % 2 == 0:
                        nc.scalar.copy(rn[:], rf[:])
                    else:
                        nc.vector.tensor_copy(out=rn[:], in_=rf[:])
                    rns.append(rn)
                filler(0)

                # broadcast 1/norm over each head's 64 partitions; divide q'
                q2s = []
                for p in range(4):
                    bc = pmid.tile([128, BLK], F32, name="ps_bc", tag="mid")
                    nc.tensor.matmul(
                        bc[:], oh2_sb[:], rns[p][:], start=True, stop=True
                    )
                    q2 = small.tile([128, BLK], BF16, name="q2", tag="q2", bufs=6)
                    nc.vector.tensor_mul(q2[:], qp_sb[:, p, :], bc[:])
                    q2s.append(q2)
                filler(1)

                o_sb = work.tile([128, 4, BLK], BF16, name="o_sb", tag="o", bufs=3)
                for p in range(4):
                    po = pmid.tile([128, BLK], F32, name="ps_o", tag="mid")
                    nc.tensor.matmul(
                        po[:], kvbd[:, p, :], q2s[p][:], start=True, stop=True
                    )
                    if p % 2 == 0:
                        nc.scalar.copy(o_sb[:, p, :], po[:])
                    else:
                        nc.vector.tensor_copy(out=o_sb[:, p, :], in_=po[:])
                filler(2)
                filler(3)

                # flush the previous block's pending proj, keep ours pending
                if prev_o is not None:
                    emit_pj(prev_o[0], prev_o[1], range(8))
                prev_o = (blk, o_sb)

                if have_next:
                    cur = nxt

            # proj of the final block
            emit_pj(prev_o[0], prev_o[1], range(8))

            pmid.release()

    nc.compile()
    return nc


_NC = None


def _get_nc():
    global _NC
    if _NC is None:
        _NC = _build_nc()
    return _NC


def _host_inputs(x, W_qkv, b_qkv, W_proj, b_proj, proj_mat):
    x = np.asarray(x, dtype=np.float32)
    W_qkv = np.asarray(W_qkv, dtype=np.float32)
    b_qkv = np.asarray(b_qkv, dtype=np.float32)
    W_proj = np.asarray(W_proj, dtype=np.float32)
    proj_mat = np.asarray(proj_mat, dtype=np.float32)

    pt = (proj_mat.T * SCALE).astype(np.float32)  # [hd, F]
    oh2 = np.zeros((2, 128), dtype=np.float32)
    oh2[0, :64] = 1.0
    oh2[1, 64:] = 1.0

    xts = [np.ascontiguousarray(x[b].T).astype(ml_dtypes.bfloat16) for b in range(4)]

    def fuse(Wslc, bslc):
        # W_fused[:, (h f)] = sum_d W.T[:, (h d)] pt[d, f]; bias likewise
        wT = Wslc.T.reshape(D, 8, HD)
        wf = np.einsum("ahd,df->ahf", wT, pt).reshape(D, 512)
        bf = np.einsum("hd,df->hf", bslc.reshape(8, HD), pt).reshape(512)
        return wf, bf

    in_maps = []
    for c in range(8):
        b, g = c // 2, c % 2
        wqs = W_qkv[g * 512 : (g + 1) * 512]
        wks = W_qkv[D + g * 512 : D + (g + 1) * 512]
        wvs = W_qkv[2 * D + g * 512 : 2 * D + (g + 1) * 512]
        bqs = b_qkv[g * 512 : (g + 1) * 512]
        bks = b_qkv[D + g * 512 : D + (g + 1) * 512]
        bvs = b_qkv[2 * D + g * 512 : 2 * D + (g + 1) * 512]
        wqp, bqp = fuse(wqs, bqs)
        wkp, bkp = fuse(wks, bks)
        bvb = np.empty((128, 4, 64), dtype=np.float32)
        bv_r = bvs.reshape(4, 2, 64)
        for p in range(4):
            bvb[0:64, p, :] = bv_r[p, 0][None, :]
            bvb[64:128, p, :] = bv_r[p, 1][None, :]
        in_maps.append(
            {
                "xt": xts[b],
                "wqp": np.ascontiguousarray(wqp).astype(ml_dtypes.bfloat16),
                "wkp": np.ascontiguousarray(wkp).astype(ml_dtypes.bfloat16),
                "wv": np.ascontiguousarray(wvs.T).astype(ml_dtypes.bfloat16),
                "wp": np.ascontiguousarray(
                    W_proj[:, g * 512 : (g + 1) * 512].T
                ).astype(ml_dtypes.bfloat16),
                "bqpe": np.ascontiguousarray(
                    (bqp + EPS).reshape(4, 128).T
                ).astype(np.float32),
                "bkp": bkp.reshape(1, 512).astype(ml_dtypes.bfloat16),
                "bvb": bvb,
                "oh2": oh2.astype(ml_dtypes.bfloat16),
            }
        )
    return in_maps


def kernel(x, W_qkv, b_qkv, W_proj, b_proj, proj_mat):
    b_proj = np.asarray(b_proj, dtype=np.float32)
    in_maps = _host_inputs(x, W_qkv, b_qkv, W_proj, b_proj, proj_mat)
    nc = _get_nc()
    res = run_bass_kernel_spmd(nc, in_maps, core_ids=list(range(8)))
    final = np.empty((4, N, D), dtype=np.float32)
    for b in range(4):
        acc = res.results[2 * b]["out"] + res.results[2 * b + 1]["out"]
        final[b] = acc.T + b_proj[None, :]
    return final


# revision 27
# speedup vs baseline: 1.0603x; 1.0151x over previous
"""FAVOR+ attention (Performer) Trainium2 kernel, 8-way sharded.

Sharding: 8 cores = 4 batches x 2 head-groups. Core c handles batch c//2 and
heads [8*(c%2), 8*(c%2)+8). The attention core (kv state) is fully local per
head; the output projection is computed as a per-core partial over its 512
input channels and the two partials per batch are summed on the host.

All matmuls run in bf16 (1 cycle/row on the PE); accumulation is fp32 in
PSUM. The output is bias-dominated, so bf16 operand rounding keeps the final
relative error at the few-1e-3 level.
"""

import numpy as np
import ml_dtypes

import concourse.mybir as mybir
import concourse.tile as tile
from concourse import bacc
from concourse.bass_utils import run_bass_kernel_spmd

F32 = mybir.dt.float32
BF16 = mybir.dt.bfloat16
AF = mybir.ActivationFunctionType
ALU = mybir.AluOpType

N = 4096
D = 1024
HD = 64
NF = 64
EPS = 1e-4
BLK = 512  # n-block
NBLK = N // BLK
NCH = BLK // 128  # 128-row chunks per block
SCALE = float(HD) ** -0.25


def _build_nc():
    nc = bacc.Bacc("TRN2", target_bir_lowering=False, debug=False, num_devices=8)

    xt = nc.dram_tensor("xt", [D, N], BF16, kind="ExternalInput").ap()
    wqp = nc.dram_tensor("wqp", [D, 512], BF16, kind="ExternalInput").ap()
    wkp = nc.dram_tensor("wkp", [D, 512], BF16, kind="ExternalInput").ap()
    wv = nc.dram_tensor("wv", [D, 512], BF16, kind="ExternalInput").ap()
    wp = nc.dram_tensor("wp", [512, D], BF16, kind="ExternalInput").ap()
    bqpe = nc.dram_tensor("bqpe", [128, 4], F32, kind="ExternalInput").ap()
    bkp = nc.dram_tensor("bkp", [1, 512], BF16, kind="ExternalInput").ap()
    bvb = nc.dram_tensor("bvb", [128, 4, 64], F32, kind="ExternalInput").ap()
    oh2 = nc.dram_tensor("oh2", [2, 128], BF16, kind="ExternalInput").ap()
    out = nc.dram_tensor("out", [D, N], F32, kind="ExternalOutput").ap()

    xt_v = xt.rearrange("(dc p) n -> p dc n", p=128)  # [128, 8, 4096]
    wqp_v = wqp.rearrange("(dc p) j -> p dc j", p=128)  # [128, 8, 512]
    wkp_v = wkp.rearrange("(dc p) j -> p dc j", p=128)
    wv_v = wv.rearrange("(dc p) j -> p dc j", p=128)
    wp_v = wp.rearrange("(jc p) o -> p jc o", p=128)  # [128, 4, 1024]
    out_v = out.rearrange("(oc p) n -> p oc n", p=128)  # [128, 8, 4096]

    with tile.TileContext(nc) as tc:
        with (
            tc.tile_pool(name="consts", bufs=1) as consts,
            tc.tile_pool(name="xp", bufs=4) as xp,
            tc.tile_pool(name="work", bufs=2) as work,
            tc.tile_pool(name="small", bufs=4) as small,
            tc.tile_pool(name="pbig", bufs=6, space="PSUM") as pbig,
        ):
            pkv = tc.alloc_tile_pool(name="pkv", bufs=1, space="PSUM")
            # ---- constants / weights (pass-A-critical loads first) ----
            wv_sb = consts.tile([128, 8, 512], BF16, name="wv_sb")
            nc.scalar.dma_start(wv_sb[:, 0:4, :], wv_v[:, 0:4, :])
            wkp_sb = consts.tile([128, 8, 512], BF16, name="wkp_sb")
            nc.gpsimd.dma_start(wkp_sb[:, 0:4, :], wkp_v[:, 0:4, :])
            bkp_sb = consts.tile([1, 512], BF16, name="bkp_sb")
            nc.gpsimd.dma_start(bkp_sb[:], bkp)
            nc.scalar.dma_start(wv_sb[:, 4:8, :], wv_v[:, 4:8, :])
            nc.gpsimd.dma_start(wkp_sb[:, 4:8, :], wkp_v[:, 4:8, :])
            eps_sb = consts.tile([128, 1], F32, name="eps_sb")
            nc.vector.memset(eps_sb[:], EPS)
            ones1_sb = consts.tile([1, 128], BF16, name="ones1_sb")
            nc.vector.memset(ones1_sb[:], 1.0)

            # declared now, loaded during pass A
            wqp_sb = consts.tile([128, 8, 512], BF16, name="wqp_sb")
            wp_sb = consts.tile([128, 4, 1024], BF16, name="wp_sb")
            oh2_sb = consts.tile([2, 128], BF16, name="oh2_sb")
            bqpe_sb = consts.tile([128, 4], F32, name="bqpe_sb")
            bvb_sb = consts.tile([128, 4, 64], F32, name="bvb_sb")

            # kv accumulators: pairs (0,1) in kvacc0, (2,3) in kvacc1.
            # Layout per pair: 129 cols (64 v-head0 | 64 v-head1 | ksum), stride 130.
            kvacc = [
                pkv.tile([128, 260], F32, name=f"kvacc{t}", tag=f"kvacc{t}")
                for t in range(2)
            ]

            # ================= pass A: k', v -> kv, ksum =================
            for blk in range(NBLK):
                ns = slice(blk * BLK, (blk + 1) * BLK)
                xt_t = xp.tile([128, 8, BLK], BF16, name="xt_t", tag="xt")
                if blk == 0:
                    nc.sync.dma_start(xt_t[:, 0:4, :], xt_v[:, 0:4, ns])
                    nc.sync.dma_start(xt_t[:, 4:8, :], xt_v[:, 4:8, ns])
                else:
                    nc.sync.dma_start(xt_t[:], xt_v[:, :, ns])

                # v and k-features (x @ Wkp + bkp, fused on host) per chunk
                v_sbs = []
                kp_sbs = []
                for c in range(NCH):
                    cs = slice(c * 128, (c + 1) * 128)
                    psv = pbig.tile([128, 512], F32, name="ps_v", tag="big")
                    for dc in range(8):
                        nc.tensor.matmul(
                            psv[:],
                            xt_t[:, dc, cs],
                            wv_sb[:, dc, :],
                            start=(dc == 0),
                            stop=(dc == 7),
                        )
                    v_sb = work.tile([128, 4, 132], BF16, name="v_sb", tag="v", bufs=5)
                    nc.scalar.copy(
                        v_sb[:, :, 0:128],
                        psv.rearrange("p (g j) -> p g j", j=128),
                    )
                    nc.vector.memset(v_sb[:, :, 128:129], 1.0)
                    v_sbs.append(v_sb)

                    psf = pbig.tile([128, 512], F32, name="ps_kf", tag="big")
                    for dc in range(8):
                        nc.tensor.matmul(
                            psf[:],
                            xt_t[:, dc, cs],
                            wkp_sb[:, dc, :],
                            start=(dc == 0),
                            stop=False,
                        )
                    nc.tensor.matmul(
                        psf[:], ones1_sb[:], bkp_sb[:], start=False, stop=True
                    )
                    psf_v = psf.rearrange("p (g f) -> p g f", f=64)  # [128, 8, 64]
                    mx = small.tile([128, 8], F32, name="mx", tag="mx")
                    nc.vector.reduce_max(mx[:], psf_v, axis=mybir.AxisListType.X)
                    karg = small.tile([128, 8, 64], F32, name="karg", tag="karg")
                    nc.vector.tensor_tensor(
                        karg[:], psf_v,
                        mx[:, :, None].to_broadcast([128, 8, 64]),
                        ALU.subtract,
                    )
                    kp_sb = work.tile([128, 4, 128], BF16, name="kp_sb", tag="kp", bufs=5)
                    nc.scalar.activation(
                        kp_sb.rearrange("p g (h f) -> p (g h) f", f=64),
                        karg[:], AF.Exp, bias=eps_sb[:], scale=1.0,
                    )
                    kp_sbs.append(kp_sb)

                # kv (+ksum) accumulation
                for c in range(NCH):
                    glob_first = blk == 0 and c == 0
                    glob_last = blk == NBLK - 1 and c == NCH - 1
                    for p in range(4):
                        base = (p % 2) * 130
                        nc.tensor.matmul(
                            kvacc[p // 2][:, base : base + 129],
                            kp_sbs[c][:, p, :],
                            v_sbs[c][:, p, 0:129],
                            start=(glob_first and p % 2 == 0),
                            stop=(glob_last and p % 2 == 1),
                        )

                if blk == 0:
                    # stream pass-B weights while pass A computes (gpsimd
                    # SWDGE queue; the sync queue keeps feeding xt blocks)
                    nc.gpsimd.dma_start(wqp_sb[:], wqp_v)
                    nc.gpsimd.dma_start(wp_sb[:], wp_v)
                    nc.gpsimd.dma_start(oh2_sb[:], oh2)
                    nc.gpsimd.dma_start(bqpe_sb[:], bqpe)
                    nc.gpsimd.dma_start(bvb_sb[:], bvb)

            # ============ assemble kv blockdiag + ksum columns ============
            kvbd = consts.tile([128, 4, 128], BF16, name="kvbd")
            ksbc = consts.tile([128, 4, 2], BF16, name="ksbc")
            nc.vector.memset(kvbd[:], 0.0)
            nc.vector.memset(ksbc[:], 0.0)
            for p in range(4):
                t = kvacc[p // 2]
                base = (p % 2) * 130
                ks = t[:, base + 128 : base + 129]
                nc.vector.tensor_copy(out=ksbc[0:64, p, 0:1], in_=ks[0:64])
                nc.vector.tensor_copy(out=ksbc[64:128, p, 1:2], in_=ks[64:128])
                # kv[h] += ksum[h] (x) bv[h], fold v-bias into kv
                nc.vector.scalar_tensor_tensor(
                    out=kvbd[0:64, p, 0:64],
                    in0=bvb_sb[0:64, p, :],
                    scalar=ks[0:64],
                    in1=t[0:64, base : base + 64],
                    op0=ALU.mult,
                    op1=ALU.add,
                )
                nc.vector.scalar_tensor_tensor(
                    out=kvbd[64:128, p, 64:128],
                    in0=bvb_sb[64:128, p, :],
                    scalar=ks[64:128],
                    in1=t[64:128, base + 64 : base + 128],
                    op0=ALU.mult,
                    op1=ALU.add,
                )

            # kv accumulator banks are dead now; reuse them for the
            # normalizer tiles of pass B.
            pkv.release()
            pnrm = tc.alloc_tile_pool(name="pnrm", bufs=2, space="PSUM")

            # ================= pass B: q', out, proj =================
            # q_proj^T comes straight from x @ Wqp (feature projection fused
            # into the weights on the host); exp bias carries bqp + eps.
            # The nrm -> bc -> po chain has an ACT/DVE hop between stages;
            # interleave each stage with one q-projection group of the NEXT
            # block so the PE never idles (keeps HAM at 2.4 GHz).
            def emit_qp_start(blk):
                ns = slice(blk * BLK, (blk + 1) * BLK)
                xt_t = xp.tile([128, 8, BLK], BF16, name="xt_t2", tag="xt")
                nc.sync.dma_start(xt_t[:], xt_v[:, :, ns])
                qp_sb = work.tile([128, 4, BLK], BF16, name="qp_sb", tag="qp")
                return xt_t, qp_sb

            def emit_qp_group(xt_t, qp_sb, p):
                ps = pbig.tile([128, BLK], F32, name="ps_qt", tag="big")
                for dc in range(8):
                    nc.tensor.matmul(
                        ps[:],
                        wqp_sb[:, dc, p * 128 : (p + 1) * 128],
                        xt_t[:, dc, :],
                        start=(dc == 0),
                        stop=(dc == 7),
                    )
                nc.scalar.activation(
                    qp_sb[:, p, :], ps[:], AF.Exp,
                    bias=bqpe_sb[:, p : p + 1], scale=1.0,
                )

            def emit_pj(blk, o_sb, oc_range):
                ns = slice(blk * BLK, (blk + 1) * BLK)
                for oc in oc_range:
                    pj = pbig.tile([128, BLK], F32, name="ps_pj", tag="big")
                    for jc in range(4):
                        nc.tensor.matmul(
                            pj[:],
                            wp_sb[:, jc, oc * 128 : (oc + 1) * 128],
                            o_sb[:, jc, :],
                            start=(jc == 0),
                            stop=(jc == 3),
                        )
                    pj_sb = small.tile([128, BLK], F32, name="pj_sb", tag="pj", bufs=6)
                    if oc % 2 == 0:
                        nc.vector.tensor_copy(out=pj_sb[:], in_=pj[:])
                    else:
                        nc.scalar.copy(pj_sb[:], pj[:])
                    nc.sync.dma_start(out_v[:, oc, ns], pj_sb[:])

            cur = emit_qp_start(0)
            for p in range(4):
                emit_qp_group(cur[0], cur[1], p)

            prev_o = None  # (blk, o_sb) whose proj is still pending
            for blk in range(NBLK):
                qp_sb = cur[1]
                have_next = blk + 1 < NBLK
                if have_next:
                    nxt = emit_qp_start(blk + 1)

                    def filler(stage):
                        emit_qp_group(nxt[0], nxt[1], stage)
                else:
                    # last block: no next q-projection; fill with pending proj
                    lo_blk, lo_sb = prev_o
                    prev_o = None

                    def filler(stage):
                        emit_pj(lo_blk, lo_sb, range(stage * 2, stage * 2 + 2))

                # normalizer -> 1/norm (approx, 18 bits) -> bf16
                rns = []
                for p in range(4):
                    nrm = pnrm.tile([2, BLK], F32, name="nrm", tag="nrm")
                    nc.tensor.matmul(
                        nrm[:], ksbc[:, p, :], qp_sb[:, p, :],
                        start=True, stop=True,
                    )
                    rf = small.tile([2, BLK], F32, name="rf", tag="rf")
                    nc.vector.reciprocal_approx_fast(out=rf[:], in_=nrm[:])
                    rn = small.tile([2, BLK], BF16, name="rn", tag="rn")
                    if p % 2 == 0:
                        nc.scalar.copy(rn[:], rf[:])
                    else:
                        nc.vector.tensor_copy(out=rn[:], in_=rf[:])
                    rns.append(rn)
                filler(0)

                # broadcast 1/norm over each head's 64 partitions; divide q'
                q2s = []
                for p in range(4):
                    bc = pbig.tile([128, BLK], F32, name="ps_bc", tag="big")
                    nc.tensor.matmul(
                        bc[:], oh2_sb[:], rns[p][:], start=True, stop=True
                    )
                    q2 = small.tile([128, BLK], BF16, name="q2", tag="q2", bufs=6)
                    nc.vector.tensor_mul(q2[:], qp_sb[:, p, :], bc[:])
                    q2s.append(q2)
                filler(1)

                o_sb = work.tile([128, 4, BLK], BF16, name="o_sb", tag="o", bufs=3)
                for p in range(4):
                    po = pbig.tile([128, BLK], F32, name="ps_o", tag="big")
                    nc.tensor.matmul(
                        po[:], kvbd[:, p, :], q2s[p][:], start=True, stop=True
                    )
                    if p % 2 == 0:
                        nc.scalar.copy(o_sb[:, p, :], po[:])
                    else:
                        nc.vector.tensor_copy(out=o_sb[:, p, :], in_=po[:])
                filler(2)
                filler(3)

                # flush the previous block's pending proj, keep ours pending
                if prev_o is not None:
                    emit_pj(prev_o[0], prev_o[1], range(8))
                prev_o = (blk, o_sb)

                if have_next:
                    cur = nxt

            # proj of the final block
            emit_pj(prev_o[0], prev_o[1], range(8))

            pnrm.release()

    nc.compile()
    return nc


_NC = None


def _get_nc():
    global _NC
    if _NC is None:
        _NC = _build_nc()
    return _NC


def _host_inputs(x, W_qkv, b_qkv, W_proj, b_proj, proj_mat):
    x = np.asarray(x, dtype=np.float32)
    W_qkv = np.asarray(W_qkv, dtype=np.float32)
    b_qkv = np.asarray(b_qkv, dtype=np.float32)
    W_proj = np.asarray(W_proj, dtype=np.float32)
    proj_mat = np.asarray(proj_mat, dtype=np.float32)

    pt = (proj_mat.T * SCALE).astype(np.float32)  # [hd, F]
    oh2 = np.zeros((2, 128), dtype=np.float32)
    oh2[0, :64] = 1.0
    oh2[1, 64:] = 1.0

    xts = [np.ascontiguousarray(x[b].T).astype(ml_dtypes.bfloat16) for b in range(4)]

    def fuse(Wslc, bslc):
        # W_fused[:, (h f)] = sum_d W.T[:, (h d)] pt[d, f]; bias likewise
        wT = Wslc.T.reshape(D, 8, HD)
        wf = np.einsum("ahd,df->ahf", wT, pt).reshape(D, 512)
        bf = np.einsum("hd,df->hf", bslc.reshape(8, HD), pt).reshape(512)
        return wf, bf

    in_maps = []
    for c in range(8):
        b, g = c // 2, c % 2
        wqs = W_qkv[g * 512 : (g + 1) * 512]
        wks = W_qkv[D + g * 512 : D + (g + 1) * 512]
        wvs = W_qkv[2 * D + g * 512 : 2 * D + (g + 1) * 512]
        bqs = b_qkv[g * 512 : (g + 1) * 512]
        bks = b_qkv[D + g * 512 : D + (g + 1) * 512]
        bvs = b_qkv[2 * D + g * 512 : 2 * D + (g + 1) * 512]
        wqp, bqp = fuse(wqs, bqs)
        wkp, bkp = fuse(wks, bks)
        bvb = np.empty((128, 4, 64), dtype=np.float32)
        bv_r = bvs.reshape(4, 2, 64)
        for p in range(4):
            bvb[0:64, p, :] = bv_r[p, 0][None, :]
            bvb[64:128, p, :] = bv_r[p, 1][None, :]
        in_maps.append(
            {
                "xt": xts[b],
                "wqp": np.ascontiguousarray(wqp).astype(ml_dtypes.bfloat16),
                "wkp": np.ascontiguousarray(wkp).astype(ml_dtypes.bfloat16),
                "wv": np.ascontiguousarray(wvs.T).astype(ml_dtypes.bfloat16),
                "wp": np.ascontiguousarray(
                    W_proj[:, g * 512 : (g + 1) * 512].T
                ).astype(ml_dtypes.bfloat16),
                "bqpe": np.ascontiguousarray(
                    (bqp + EPS).reshape(4, 128).T
                ).astype(np.float32),
                "bkp": bkp.reshape(1, 512).astype(ml_dtypes.bfloat16),
                "bvb": bvb,
                "oh2": oh2.astype(ml_dtypes.bfloat16),
            }
        )
    return in_maps


def kernel(x, W_qkv, b_qkv, W_proj, b_proj, proj_mat):
    b_proj = np.asarray(b_proj, dtype=np.float32)
    in_maps = _host_inputs(x, W_qkv, b_qkv, W_proj, b_proj, proj_mat)
    nc = _get_nc()
    res = run_bass_kernel_spmd(nc, in_maps, core_ids=list(range(8)))
    final = np.empty((4, N, D), dtype=np.float32)
    for b in range(4):
        acc = res.results[2 * b]["out"] + res.results[2 * b + 1]["out"]
        final[b] = acc.T + b_proj[None, :]
    return final


# revision 28
# speedup vs baseline: 1.0953x; 1.0330x over previous
"""FAVOR+ attention (Performer) Trainium2 kernel, 8-way sharded.

Sharding: 8 cores = 4 batches x 2 head-groups. Core c handles batch c//2 and
heads [8*(c%2), 8*(c%2)+8). The attention core (kv state) is fully local per
head; the output projection is computed as a per-core partial over its 512
input channels and the two partials per batch are summed on the host.

All matmuls run in bf16 (1 cycle/row on the PE); accumulation is fp32 in
PSUM. The output is bias-dominated, so bf16 operand rounding keeps the final
relative error at the few-1e-3 level.
"""

import numpy as np
import ml_dtypes

import concourse.mybir as mybir
import concourse.tile as tile
from concourse import bacc
from concourse.bass_utils import run_bass_kernel_spmd

F32 = mybir.dt.float32
BF16 = mybir.dt.bfloat16
AF = mybir.ActivationFunctionType
ALU = mybir.AluOpType

N = 4096
D = 1024
HD = 64
NF = 64
EPS = 1e-4
BLK = 512  # n-block
NBLK = N // BLK
NCH = BLK // 128  # 128-row chunks per block
SCALE = float(HD) ** -0.25


def _build_nc():
    nc = bacc.Bacc("TRN2", target_bir_lowering=False, debug=False, num_devices=8)

    xt = nc.dram_tensor("xt", [D, N], BF16, kind="ExternalInput").ap()
    wqp = nc.dram_tensor("wqp", [D, 512], BF16, kind="ExternalInput").ap()
    wkp = nc.dram_tensor("wkp", [D, 512], BF16, kind="ExternalInput").ap()
    wv = nc.dram_tensor("wv", [D, 512], BF16, kind="ExternalInput").ap()
    wp = nc.dram_tensor("wp", [512, D], BF16, kind="ExternalInput").ap()
    bqpe = nc.dram_tensor("bqpe", [128, 4], F32, kind="ExternalInput").ap()
    bkpb = nc.dram_tensor("bkpb", [128, 8, 64], F32, kind="ExternalInput").ap()
    bvb = nc.dram_tensor("bvb", [128, 4, 64], F32, kind="ExternalInput").ap()
    oh2 = nc.dram_tensor("oh2", [2, 128], BF16, kind="ExternalInput").ap()
    out = nc.dram_tensor("out", [D, N], F32, kind="ExternalOutput").ap()

    xt_v = xt.rearrange("(dc p) n -> p dc n", p=128)  # [128, 8, 4096]
    wqp_v = wqp.rearrange("(dc p) j -> p dc j", p=128)  # [128, 8, 512]
    wkp_v = wkp.rearrange("(dc p) j -> p dc j", p=128)
    wv_v = wv.rearrange("(dc p) j -> p dc j", p=128)
    wp_v = wp.rearrange("(jc p) o -> p jc o", p=128)  # [128, 4, 1024]
    out_v = out.rearrange("(oc p) n -> p oc n", p=128)  # [128, 8, 4096]

    with tile.TileContext(nc) as tc:
        with (
            tc.tile_pool(name="consts", bufs=1) as consts,
            tc.tile_pool(name="xp", bufs=4) as xp,
            tc.tile_pool(name="work", bufs=2) as work,
            tc.tile_pool(name="small", bufs=4) as small,
            tc.tile_pool(name="pbig", bufs=6, space="PSUM") as pbig,
        ):
            pkv = tc.alloc_tile_pool(name="pkv", bufs=1, space="PSUM")
            # ---- constants / weights (pass-A-critical loads first) ----
            wv_sb = consts.tile([128, 8, 512], BF16, name="wv_sb")
            nc.scalar.dma_start(wv_sb[:, 0:4, :], wv_v[:, 0:4, :])
            wkp_sb = consts.tile([128, 8, 512], BF16, name="wkp_sb")
            bkpb_sb = consts.tile([128, 8, 64], F32, name="bkpb_sb")
            nc.gpsimd.dma_start(bkpb_sb[:], bkpb)
            nc.gpsimd.dma_start(wkp_sb[:, 0:4, :], wkp_v[:, 0:4, :])
            nc.scalar.dma_start(wv_sb[:, 4:8, :], wv_v[:, 4:8, :])
            nc.gpsimd.dma_start(wkp_sb[:, 4:8, :], wkp_v[:, 4:8, :])
            eps_sb = consts.tile([128, 1], F32, name="eps_sb")
            nc.vector.memset(eps_sb[:], EPS)

            # declared now, loaded during pass A
            wqp_sb = consts.tile([128, 8, 512], BF16, name="wqp_sb")
            wp_sb = consts.tile([128, 4, 1024], BF16, name="wp_sb")
            oh2_sb = consts.tile([2, 128], BF16, name="oh2_sb")
            bqpe_sb = consts.tile([128, 4], F32, name="bqpe_sb")
            bvb_sb = consts.tile([128, 4, 64], F32, name="bvb_sb")

            # kv accumulators: pairs (0,1) in kvacc0, (2,3) in kvacc1.
            # Layout per pair: 129 cols (64 v-head0 | 64 v-head1 | ksum), stride 130.
            kvacc = [
                pkv.tile([128, 260], F32, name=f"kvacc{t}", tag=f"kvacc{t}")
                for t in range(2)
            ]

            # ================= pass A: k', v -> kv, ksum =================
            for blk in range(NBLK):
                ns = slice(blk * BLK, (blk + 1) * BLK)
                xt_t = xp.tile([128, 8, BLK], BF16, name="xt_t", tag="xt")
                if blk == 0:
                    nc.sync.dma_start(xt_t[:, 0:4, :], xt_v[:, 0:4, ns])
                    nc.sync.dma_start(xt_t[:, 4:8, :], xt_v[:, 4:8, ns])
                else:
                    nc.sync.dma_start(xt_t[:], xt_v[:, :, ns])

                # v and k-features (x @ Wkp + bkp, fused on host) per chunk
                v_sbs = []
                kp_sbs = []
                for c in range(NCH):
                    cs = slice(c * 128, (c + 1) * 128)
                    psv = pbig.tile([128, 512], F32, name="ps_v", tag="big")
                    for dc in range(8):
                        nc.tensor.matmul(
                            psv[:],
                            xt_t[:, dc, cs],
                            wv_sb[:, dc, :],
                            start=(dc == 0),
                            stop=(dc == 7),
                        )
                    v_sb = work.tile([128, 4, 132], BF16, name="v_sb", tag="v", bufs=5)
                    nc.scalar.copy(
                        v_sb[:, :, 0:128],
                        psv.rearrange("p (g j) -> p g j", j=128),
                    )
                    nc.vector.memset(v_sb[:, :, 128:129], 1.0)
                    v_sbs.append(v_sb)

                    psf = pbig.tile([128, 512], F32, name="ps_kf", tag="big")
                    for dc in range(8):
                        nc.tensor.matmul(
                            psf[:],
                            xt_t[:, dc, cs],
                            wkp_sb[:, dc, :],
                            start=(dc == 0),
                            stop=(dc == 7),
                        )
                    psf_v = psf.rearrange("p (g f) -> p g f", f=64)  # [128, 8, 64]
                    karg = small.tile([128, 8, 64], F32, name="karg", tag="karg")
                    nc.vector.tensor_tensor(karg[:], psf_v, bkpb_sb[:], ALU.add)
                    mx = small.tile([128, 8], F32, name="mx", tag="mx")
                    nc.vector.reduce_max(mx[:], karg[:], axis=mybir.AxisListType.X)
                    nc.vector.tensor_tensor(
                        karg[:], karg[:],
                        mx[:, :, None].to_broadcast([128, 8, 64]),
                        ALU.subtract,
                    )
                    kp_sb = work.tile([128, 4, 128], BF16, name="kp_sb", tag="kp", bufs=5)
                    nc.scalar.activation(
                        kp_sb.rearrange("p g (h f) -> p (g h) f", f=64),
                        karg[:], AF.Exp, bias=eps_sb[:], scale=1.0,
                    )
                    kp_sbs.append(kp_sb)

                # kv (+ksum) accumulation
                for c in range(NCH):
                    glob_first = blk == 0 and c == 0
                    glob_last = blk == NBLK - 1 and c == NCH - 1
                    for p in range(4):
                        base = (p % 2) * 130
                        nc.tensor.matmul(
                            kvacc[p // 2][:, base : base + 129],
                            kp_sbs[c][:, p, :],
                            v_sbs[c][:, p, 0:129],
                            start=(glob_first and p % 2 == 0),
                            stop=(glob_last and p % 2 == 1),
                        )

                if blk == 0:
                    # stream pass-B weights while pass A computes (gpsimd
                    # SWDGE queue; the sync queue keeps feeding xt blocks)
                    nc.gpsimd.dma_start(wqp_sb[:], wqp_v)
                    nc.gpsimd.dma_start(wp_sb[:], wp_v)
                    nc.gpsimd.dma_start(oh2_sb[:], oh2)
                    nc.gpsimd.dma_start(bqpe_sb[:], bqpe)
                    nc.gpsimd.dma_start(bvb_sb[:], bvb)

            # ============ assemble kv blockdiag + ksum columns ============
            kvbd = consts.tile([128, 4, 128], BF16, name="kvbd")
            ksbc = consts.tile([128, 4, 2], BF16, name="ksbc")
            nc.vector.memset(kvbd[:], 0.0)
            nc.vector.memset(ksbc[:], 0.0)
            for p in range(4):
                t = kvacc[p // 2]
                base = (p % 2) * 130
                ks = t[:, base + 128 : base + 129]
                nc.vector.tensor_copy(out=ksbc[0:64, p, 0:1], in_=ks[0:64])
                nc.vector.tensor_copy(out=ksbc[64:128, p, 1:2], in_=ks[64:128])
                # kv[h] += ksum[h] (x) bv[h], fold v-bias into kv
                nc.vector.scalar_tensor_tensor(
                    out=kvbd[0:64, p, 0:64],
                    in0=bvb_sb[0:64, p, :],
                    scalar=ks[0:64],
                    in1=t[0:64, base : base + 64],
                    op0=ALU.mult,
                    op1=ALU.add,
                )
                nc.vector.scalar_tensor_tensor(
                    out=kvbd[64:128, p, 64:128],
                    in0=bvb_sb[64:128, p, :],
                    scalar=ks[64:128],
                    in1=t[64:128, base + 64 : base + 128],
                    op0=ALU.mult,
                    op1=ALU.add,
                )

            # kv accumulator banks are dead now; reuse them for the
            # normalizer tiles of pass B.
            pkv.release()
            pnrm = tc.alloc_tile_pool(name="pnrm", bufs=2, space="PSUM")

            # ================= pass B: q', out, proj =================
            # q_proj^T comes straight from x @ Wqp (feature projection fused
            # into the weights on the host); exp bias carries bqp + eps.
            # The nrm -> bc -> po chain has an ACT/DVE hop between stages;
            # interleave each stage with one q-projection group of the NEXT
            # block so the PE never idles (keeps HAM at 2.4 GHz).
            def emit_qp_start(blk):
                ns = slice(blk * BLK, (blk + 1) * BLK)
                xt_t = xp.tile([128, 8, BLK], BF16, name="xt_t2", tag="xt")
                nc.sync.dma_start(xt_t[:], xt_v[:, :, ns])
                qp_sb = work.tile([128, 4, BLK], BF16, name="qp_sb", tag="qp")
                return xt_t, qp_sb

            def emit_qp_group(xt_t, qp_sb, p):
                ps = pbig.tile([128, BLK], F32, name="ps_qt", tag="big")
                for dc in range(8):
                    nc.tensor.matmul(
                        ps[:],
                        wqp_sb[:, dc, p * 128 : (p + 1) * 128],
                        xt_t[:, dc, :],
                        start=(dc == 0),
                        stop=(dc == 7),
                    )
                nc.scalar.activation(
                    qp_sb[:, p, :], ps[:], AF.Exp,
                    bias=bqpe_sb[:, p : p + 1], scale=1.0,
                )

            def emit_pj(blk, o_sb, oc_range):
                ns = slice(blk * BLK, (blk + 1) * BLK)
                for oc in oc_range:
                    pj = pbig.tile([128, BLK], F32, name="ps_pj", tag="big")
                    for jc in range(4):
                        nc.tensor.matmul(
                            pj[:],
                            wp_sb[:, jc, oc * 128 : (oc + 1) * 128],
                            o_sb[:, jc, :],
                            start=(jc == 0),
                            stop=(jc == 3),
                        )
                    pj_sb = small.tile([128, BLK], F32, name="pj_sb", tag="pj", bufs=6)
                    if oc % 2 == 0:
                        nc.vector.tensor_copy(out=pj_sb[:], in_=pj[:])
                    else:
                        nc.scalar.copy(pj_sb[:], pj[:])
                    nc.sync.dma_start(out_v[:, oc, ns], pj_sb[:])

            cur = emit_qp_start(0)
            for p in range(4):
                emit_qp_group(cur[0], cur[1], p)

            prev_o = None  # (blk, o_sb) whose proj is still pending
            for blk in range(NBLK):
                qp_sb = cur[1]
                have_next = blk + 1 < NBLK
                if have_next:
                    nxt = emit_qp_start(blk + 1)

                    def filler(stage):
                        emit_qp_group(nxt[0], nxt[1], stage)
                else:
                    # last block: no next q-projection; fill with pending proj
                    lo_blk, lo_sb = prev_o
                    prev_o = None

                    def filler(stage):
                        emit_pj(lo_blk, lo_sb, range(stage * 2, stage * 2 + 2))

                # normalizer -> 1/norm (approx, 18 bits) -> bf16
                rns = []
                for p in range(4):
                    nrm = pnrm.tile([2, BLK], F32, name="nrm", tag="nrm")
                    nc.tensor.matmul(
                        nrm[:], ksbc[:, p, :], qp_sb[:, p, :],
                        start=True, stop=True,
                    )
                    rf = small.tile([2, BLK], F32, name="rf", tag="rf")
                    nc.vector.reciprocal_approx_fast(out=rf[:], in_=nrm[:])
                    rn = small.tile([2, BLK], BF16, name="rn", tag="rn")
                    if p % 2 == 0:
                        nc.scalar.copy(rn[:], rf[:])
                    else:
                        nc.vector.tensor_copy(out=rn[:], in_=rf[:])
                    rns.append(rn)
                filler(0)

                # broadcast 1/norm over each head's 64 partitions; divide q'
                q2s = []
                for p in range(4):
                    bc = pbig.tile([128, BLK], F32, name="ps_bc", tag="big")
                    nc.tensor.matmul(
                        bc[:], oh2_sb[:], rns[p][:], start=True, stop=True
                    )
                    q2 = small.tile([128, BLK], BF16, name="q2", tag="q2", bufs=6)
                    nc.vector.tensor_mul(q2[:], qp_sb[:, p, :], bc[:])
                    q2s.append(q2)
                filler(1)

                o_sb = work.tile([128, 4, BLK], BF16, name="o_sb", tag="o", bufs=3)
                for p in range(4):
                    po = pbig.tile([128, BLK], F32, name="ps_o", tag="big")
                    nc.tensor.matmul(
                        po[:], kvbd[:, p, :], q2s[p][:], start=True, stop=True
                    )
                    if p % 2 == 0:
                        nc.scalar.copy(o_sb[:, p, :], po[:])
                    else:
                        nc.vector.tensor_copy(out=o_sb[:, p, :], in_=po[:])
                filler(2)
                filler(3)

                # flush the previous block's pending proj, keep ours pending
                if prev_o is not None:
                    emit_pj(prev_o[0], prev_o[1], range(8))
                prev_o = (blk, o_sb)

                if have_next:
                    cur = nxt

            # proj of the final block
            emit_pj(prev_o[0], prev_o[1], range(8))

            pnrm.release()

    nc.compile()
    return nc


_NC = None


def _get_nc():
    global _NC
    if _NC is None:
        _NC = _build_nc()
    return _NC


def _host_inputs(x, W_qkv, b_qkv, W_proj, b_proj, proj_mat):
    x = np.asarray(x, dtype=np.float32)
    W_qkv = np.asarray(W_qkv, dtype=np.float32)
    b_qkv = np.asarray(b_qkv, dtype=np.float32)
    W_proj = np.asarray(W_proj, dtype=np.float32)
    proj_mat = np.asarray(proj_mat, dtype=np.float32)

    pt = (proj_mat.T * SCALE).astype(np.float32)  # [hd, F]
    oh2 = np.zeros((2, 128), dtype=np.float32)
    oh2[0, :64] = 1.0
    oh2[1, 64:] = 1.0

    xts = [np.ascontiguousarray(x[b].T).astype(ml_dtypes.bfloat16) for b in range(4)]

    def fuse(Wslc, bslc):
        # W_fused[:, (h f)] = sum_d W.T[:, (h d)] pt[d, f]; bias likewise
        wT = Wslc.T.reshape(D, 8, HD)
        wf = np.einsum("ahd,df->ahf", wT, pt).reshape(D, 512)
        bf = np.einsum("hd,df->hf", bslc.reshape(8, HD), pt).reshape(512)
        return wf, bf

    in_maps = []
    for c in range(8):
        b, g = c // 2, c % 2
        wqs = W_qkv[g * 512 : (g + 1) * 512]
        wks = W_qkv[D + g * 512 : D + (g + 1) * 512]
        wvs = W_qkv[2 * D + g * 512 : 2 * D + (g + 1) * 512]
        bqs = b_qkv[g * 512 : (g + 1) * 512]
        bks = b_qkv[D + g * 512 : D + (g + 1) * 512]
        bvs = b_qkv[2 * D + g * 512 : 2 * D + (g + 1) * 512]
        wqp, bqp = fuse(wqs, bqs)
        wkp, bkp = fuse(wks, bks)
        bvb = np.empty((128, 4, 64), dtype=np.float32)
        bv_r = bvs.reshape(4, 2, 64)
        for p in range(4):
            bvb[0:64, p, :] = bv_r[p, 0][None, :]
            bvb[64:128, p, :] = bv_r[p, 1][None, :]
        in_maps.append(
            {
                "xt": xts[b],
                "wqp": np.ascontiguousarray(wqp).astype(ml_dtypes.bfloat16),
                "wkp": np.ascontiguousarray(wkp).astype(ml_dtypes.bfloat16),
                "wv": np.ascontiguousarray(wvs.T).astype(ml_dtypes.bfloat16),
                "wp": np.ascontiguousarray(
                    W_proj[:, g * 512 : (g + 1) * 512].T
                ).astype(ml_dtypes.bfloat16),
                "bqpe": np.ascontiguousarray(
                    (bqp + EPS).reshape(4, 128).T
                ).astype(np.float32),
                "bkpb": np.ascontiguousarray(
                    np.broadcast_to(bkp.reshape(1, 8, 64), (128, 8, 64))
                ).astype(np.float32),
                "bvb": bvb,
                "oh2": oh2.astype(ml_dtypes.bfloat16),
            }
        )
    return in_maps


def kernel(x, W_qkv, b_qkv, W_proj, b_proj, proj_mat):
    b_proj = np.asarray(b_proj, dtype=np.float32)
    in_maps = _host_inputs(x, W_qkv, b_qkv, W_proj, b_proj, proj_mat)
    nc = _get_nc()
    res = run_bass_kernel_spmd(nc, in_maps, core_ids=list(range(8)))
    final = np.empty((4, N, D), dtype=np.float32)
    for b in range(4):
        acc = res.results[2 * b]["out"] + res.results[2 * b + 1]["out"]
        final[b] = acc.T + b_proj[None, :]
    return final


# revision 29
# speedup vs baseline: 1.0974x; 1.0019x over previous
"""FAVOR+ attention (Performer) Trainium2 kernel, 8-way sharded.

Sharding: 8 cores = 4 batches x 2 head-groups. Core c handles batch c//2 and
heads [8*(c%2), 8*(c%2)+8). The attention core (kv state) is fully local per
head; the output projection is computed as a per-core partial over its 512
input channels and the two partials per batch are summed on the host.

All matmuls run in bf16 (1 cycle/row on the PE); accumulation is fp32 in
PSUM. The output is bias-dominated, so bf16 operand rounding keeps the final
relative error at the few-1e-3 level.
"""

import numpy as np
import ml_dtypes

import concourse.mybir as mybir
import concourse.tile as tile
from concourse import bacc
from concourse.bass_utils import run_bass_kernel_spmd

F32 = mybir.dt.float32
BF16 = mybir.dt.bfloat16
AF = mybir.ActivationFunctionType
ALU = mybir.AluOpType

N = 4096
D = 1024
HD = 64
NF = 64
EPS = 1e-4
BLK = 512  # n-block
NBLK = N // BLK
NCH = BLK // 128  # 128-row chunks per block
SCALE = float(HD) ** -0.25


def _build_nc():
    nc = bacc.Bacc("TRN2", target_bir_lowering=False, debug=False, num_devices=8)

    xt = nc.dram_tensor("xt", [D, N], BF16, kind="ExternalInput").ap()
    wqp = nc.dram_tensor("wqp", [D, 512], BF16, kind="ExternalInput").ap()
    wkp = nc.dram_tensor("wkp", [D, 512], BF16, kind="ExternalInput").ap()
    wv = nc.dram_tensor("wv", [D, 512], BF16, kind="ExternalInput").ap()
    wp = nc.dram_tensor("wp", [512, D], BF16, kind="ExternalInput").ap()
    bqpe = nc.dram_tensor("bqpe", [128, 4], F32, kind="ExternalInput").ap()
    bkpb = nc.dram_tensor("bkpb", [128, 8, 64], F32, kind="ExternalInput").ap()
    bvb = nc.dram_tensor("bvb", [128, 4, 64], F32, kind="ExternalInput").ap()
    oh2 = nc.dram_tensor("oh2", [2, 128], BF16, kind="ExternalInput").ap()
    out = nc.dram_tensor("out", [D, N], F32, kind="ExternalOutput").ap()

    xt_v = xt.rearrange("(dc p) n -> p dc n", p=128)  # [128, 8, 4096]
    wqp_v = wqp.rearrange("(dc p) j -> p dc j", p=128)  # [128, 8, 512]
    wkp_v = wkp.rearrange("(dc p) j -> p dc j", p=128)
    wv_v = wv.rearrange("(dc p) j -> p dc j", p=128)
    wp_v = wp.rearrange("(jc p) o -> p jc o", p=128)  # [128, 4, 1024]
    out_v = out.rearrange("(oc p) n -> p oc n", p=128)  # [128, 8, 4096]

    with tile.TileContext(nc) as tc:
        with (
            tc.tile_pool(name="consts", bufs=1) as consts,
            tc.tile_pool(name="xp", bufs=4) as xp,
            tc.tile_pool(name="work", bufs=2) as work,
            tc.tile_pool(name="small", bufs=4) as small,
            tc.tile_pool(name="pbig", bufs=6, space="PSUM") as pbig,
        ):
            pkv = tc.alloc_tile_pool(name="pkv", bufs=1, space="PSUM")
            # ---- constants / weights (pass-A-critical loads first) ----
            wv_sb = consts.tile([128, 8, 512], BF16, name="wv_sb")
            nc.scalar.dma_start(wv_sb[:, 0:4, :], wv_v[:, 0:4, :])
            wkp_sb = consts.tile([128, 8, 512], BF16, name="wkp_sb")
            bkpb_sb = consts.tile([128, 8, 64], F32, name="bkpb_sb")
            nc.scalar.dma_start(wv_sb[:, 4:8, :], wv_v[:, 4:8, :])
            nc.scalar.dma_start(bkpb_sb[:], bkpb)
            eps_sb = consts.tile([128, 1], F32, name="eps_sb")
            nc.vector.memset(eps_sb[:], EPS)

            # declared now, loaded during pass A
            wqp_sb = consts.tile([128, 8, 512], BF16, name="wqp_sb")
            wp_sb = consts.tile([128, 4, 1024], BF16, name="wp_sb")
            oh2_sb = consts.tile([2, 128], BF16, name="oh2_sb")
            bqpe_sb = consts.tile([128, 4], F32, name="bqpe_sb")
            bvb_sb = consts.tile([128, 4, 64], F32, name="bvb_sb")

            # kv accumulators: pairs (0,1) in kvacc0, (2,3) in kvacc1.
            # Layout per pair: 129 cols (64 v-head0 | 64 v-head1 | ksum), stride 130.
            kvacc = [
                pkv.tile([128, 260], F32, name=f"kvacc{t}", tag=f"kvacc{t}")
                for t in range(2)
            ]

            # ================= pass A: k', v -> kv, ksum =================
            for blk in range(NBLK):
                ns = slice(blk * BLK, (blk + 1) * BLK)
                xt_t = xp.tile([128, 8, BLK], BF16, name="xt_t", tag="xt")
                if blk == 0:
                    nc.sync.dma_start(xt_t[:, 0:4, :], xt_v[:, 0:4, ns])
                    nc.sync.dma_start(xt_t[:, 4:8, :], xt_v[:, 4:8, ns])
                    nc.sync.dma_start(wkp_sb[:, 0:4, :], wkp_v[:, 0:4, :])
                    nc.sync.dma_start(wkp_sb[:, 4:8, :], wkp_v[:, 4:8, :])
                else:
                    nc.sync.dma_start(xt_t[:], xt_v[:, :, ns])

                # v and k-features (x @ Wkp + bkp, fused on host) per chunk
                v_sbs = []
                kp_sbs = []
                for c in range(NCH):
                    cs = slice(c * 128, (c + 1) * 128)
                    psv = pbig.tile([128, 512], F32, name="ps_v", tag="big")
                    for dc in range(8):
                        nc.tensor.matmul(
                            psv[:],
                            xt_t[:, dc, cs],
                            wv_sb[:, dc, :],
                            start=(dc == 0),
                            stop=(dc == 7),
                        )
                    v_sb = work.tile([128, 4, 132], BF16, name="v_sb", tag="v", bufs=5)
                    nc.scalar.copy(
                        v_sb[:, :, 0:128],
                        psv.rearrange("p (g j) -> p g j", j=128),
                    )
                    nc.vector.memset(v_sb[:, :, 128:129], 1.0)
                    v_sbs.append(v_sb)

                    psf = pbig.tile([128, 512], F32, name="ps_kf", tag="big")
                    for dc in range(8):
                        nc.tensor.matmul(
                            psf[:],
                            xt_t[:, dc, cs],
                            wkp_sb[:, dc, :],
                            start=(dc == 0),
                            stop=(dc == 7),
                        )
                    psf_v = psf.rearrange("p (g f) -> p g f", f=64)  # [128, 8, 64]
                    karg = small.tile([128, 8, 64], F32, name="karg", tag="karg")
                    nc.vector.tensor_tensor(karg[:], psf_v, bkpb_sb[:], ALU.add)
                    mx = small.tile([128, 8], F32, name="mx", tag="mx")
                    nc.vector.reduce_max(mx[:], karg[:], axis=mybir.AxisListType.X)
                    nc.vector.tensor_tensor(
                        karg[:], karg[:],
                        mx[:, :, None].to_broadcast([128, 8, 64]),
                        ALU.subtract,
                    )
                    kp_sb = work.tile([128, 4, 128], BF16, name="kp_sb", tag="kp", bufs=5)
                    nc.scalar.activation(
                        kp_sb.rearrange("p g (h f) -> p (g h) f", f=64),
                        karg[:], AF.Exp, bias=eps_sb[:], scale=1.0,
                    )
                    kp_sbs.append(kp_sb)

                # kv (+ksum) accumulation
                for c in range(NCH):
                    glob_first = blk == 0 and c == 0
                    glob_last = blk == NBLK - 1 and c == NCH - 1
                    for p in range(4):
                        base = (p % 2) * 130
                        nc.tensor.matmul(
                            kvacc[p // 2][:, base : base + 129],
                            kp_sbs[c][:, p, :],
                            v_sbs[c][:, p, 0:129],
                            start=(glob_first and p % 2 == 0),
                            stop=(glob_last and p % 2 == 1),
                        )

                if blk == 0:
                    # stream pass-B weights while pass A computes (gpsimd
                    # SWDGE queue; the sync queue keeps feeding xt blocks)
                    nc.gpsimd.dma_start(wqp_sb[:], wqp_v)
                    nc.gpsimd.dma_start(wp_sb[:], wp_v)
                    nc.gpsimd.dma_start(oh2_sb[:], oh2)
                    nc.gpsimd.dma_start(bqpe_sb[:], bqpe)
                    nc.gpsimd.dma_start(bvb_sb[:], bvb)

            # ============ assemble kv blockdiag + ksum columns ============
            kvbd = consts.tile([128, 4, 128], BF16, name="kvbd")
            ksbc = consts.tile([128, 4, 2], BF16, name="ksbc")
            nc.vector.memset(kvbd[:], 0.0)
            nc.vector.memset(ksbc[:], 0.0)
            for p in range(4):
                t = kvacc[p // 2]
                base = (p % 2) * 130
                ks = t[:, base + 128 : base + 129]
                nc.vector.tensor_copy(out=ksbc[0:64, p, 0:1], in_=ks[0:64])
                nc.vector.tensor_copy(out=ksbc[64:128, p, 1:2], in_=ks[64:128])
                # kv[h] += ksum[h] (x) bv[h], fold v-bias into kv
                nc.vector.scalar_tensor_tensor(
                    out=kvbd[0:64, p, 0:64],
                    in0=bvb_sb[0:64, p, :],
                    scalar=ks[0:64],
                    in1=t[0:64, base : base + 64],
                    op0=ALU.mult,
                    op1=ALU.add,
                )
                nc.vector.scalar_tensor_tensor(
                    out=kvbd[64:128, p, 64:128],
                    in0=bvb_sb[64:128, p, :],
                    scalar=ks[64:128],
                    in1=t[64:128, base + 64 : base + 128],
                    op0=ALU.mult,
                    op1=ALU.add,
                )

            # kv accumulator banks are dead now; reuse them for the
            # normalizer tiles of pass B.
            pkv.release()
            pnrm = tc.alloc_tile_pool(name="pnrm", bufs=2, space="PSUM")

            # ================= pass B: q', out, proj =================
            # q_proj^T comes straight from x @ Wqp (feature projection fused
            # into the weights on the host); exp bias carries bqp + eps.
            # The nrm -> bc -> po chain has an ACT/DVE hop between stages;
            # interleave each stage with one q-projection group of the NEXT
            # block so the PE never idles (keeps HAM at 2.4 GHz).
            def emit_qp_start(blk):
                ns = slice(blk * BLK, (blk + 1) * BLK)
                xt_t = xp.tile([128, 8, BLK], BF16, name="xt_t2", tag="xt")
                nc.sync.dma_start(xt_t[:], xt_v[:, :, ns])
                qp_sb = work.tile([128, 4, BLK], BF16, name="qp_sb", tag="qp")
                return xt_t, qp_sb

            def emit_qp_group(xt_t, qp_sb, p):
                ps = pbig.tile([128, BLK], F32, name="ps_qt", tag="big")
                for dc in range(8):
                    nc.tensor.matmul(
                        ps[:],
                        wqp_sb[:, dc, p * 128 : (p + 1) * 128],
                        xt_t[:, dc, :],
                        start=(dc == 0),
                        stop=(dc == 7),
                    )
                nc.scalar.activation(
                    qp_sb[:, p, :], ps[:], AF.Exp,
                    bias=bqpe_sb[:, p : p + 1], scale=1.0,
                )

            def emit_pj(blk, o_sb, oc_range):
                ns = slice(blk * BLK, (blk + 1) * BLK)
                for oc in oc_range:
                    pj = pbig.tile([128, BLK], F32, name="ps_pj", tag="big")
                    for jc in range(4):
                        nc.tensor.matmul(
                            pj[:],
                            wp_sb[:, jc, oc * 128 : (oc + 1) * 128],
                            o_sb[:, jc, :],
                            start=(jc == 0),
                            stop=(jc == 3),
                        )
                    pj_sb = small.tile([128, BLK], F32, name="pj_sb", tag="pj", bufs=6)
                    if oc % 2 == 0:
                        nc.vector.tensor_copy(out=pj_sb[:], in_=pj[:])
                    else:
                        nc.scalar.copy(pj_sb[:], pj[:])
                    nc.sync.dma_start(out_v[:, oc, ns], pj_sb[:])

            cur = emit_qp_start(0)
            for p in range(4):
                emit_qp_group(cur[0], cur[1], p)

            prev_o = None  # (blk, o_sb) whose proj is still pending
            for blk in range(NBLK):
                qp_sb = cur[1]
                have_next = blk + 1 < NBLK
                if have_next:
                    nxt = emit_qp_start(blk + 1)

                    def filler(stage):
                        emit_qp_group(nxt[0], nxt[1], stage)
                else:
                    # last block: no next q-projection; fill with pending proj
                    lo_blk, lo_sb = prev_o
                    prev_o = None

                    def filler(stage):
                        emit_pj(lo_blk, lo_sb, range(stage * 2, stage * 2 + 2))

                # normalizer -> 1/norm (approx, 18 bits) -> bf16
                rns = []
                for p in range(4):
                    nrm = pnrm.tile([2, BLK], F32, name="nrm", tag="nrm")
                    nc.tensor.matmul(
                        nrm[:], ksbc[:, p, :], qp_sb[:, p, :],
                        start=True, stop=True,
                    )
                    rf = small.tile([2, BLK], F32, name="rf", tag="rf")
                    nc.vector.reciprocal_approx_fast(out=rf[:], in_=nrm[:])
                    rn = small.tile([2, BLK], BF16, name="rn", tag="rn")
                    if p % 2 == 0:
                        nc.scalar.copy(rn[:], rf[:])
                    else:
                        nc.vector.tensor_copy(out=rn[:], in_=rf[:])
                    rns.append(rn)
                filler(0)

                # broadcast 1/norm over each head's 64 partitions; divide q'
                q2s = []
                for p in range(4):
                    bc = pbig.tile([128, BLK], F32, name="ps_bc", tag="big")
                    nc.tensor.matmul(
                        bc[:], oh2_sb[:], rns[p][:], start=True, stop=True
                    )
                    q2 = small.tile([128, BLK], BF16, name="q2", tag="q2", bufs=6)
                    nc.vector.tensor_mul(q2[:], qp_sb[:, p, :], bc[:])
                    q2s.append(q2)
                filler(1)

                o_sb = work.tile([128, 4, BLK], BF16, name="o_sb", tag="o", bufs=3)
                for p in range(4):
                    po = pbig.tile([128, BLK], F32, name="ps_o", tag="big")
                    nc.tensor.matmul(
                        po[:], kvbd[:, p, :], q2s[p][:], start=True, stop=True
                    )
                    if p % 2 == 0:
                        nc.scalar.copy(o_sb[:, p, :], po[:])
                    else:
                        nc.vector.tensor_copy(out=o_sb[:, p, :], in_=po[:])
                filler(2)
                filler(3)

                # flush the previous block's pending proj, keep ours pending
                if prev_o is not None:
                    emit_pj(prev_o[0], prev_o[1], range(8))
                prev_o = (blk, o_sb)

                if have_next:
                    cur = nxt

            # proj of the final block
            emit_pj(prev_o[0], prev_o[1], range(8))

            pnrm.release()

    nc.compile()
    return nc


_NC = None


def _get_nc():
    global _NC
    if _NC is None:
        _NC = _build_nc()
    return _NC


def _host_inputs(x, W_qkv, b_qkv, W_proj, b_proj, proj_mat):
    x = np.asarray(x, dtype=np.float32)
    W_qkv = np.asarray(W_qkv, dtype=np.float32)
    b_qkv = np.asarray(b_qkv, dtype=np.float32)
    W_proj = np.asarray(W_proj, dtype=np.float32)
    proj_mat = np.asarray(proj_mat, dtype=np.float32)

    pt = (proj_mat.T * SCALE).astype(np.float32)  # [hd, F]
    oh2 = np.zeros((2, 128), dtype=np.float32)
    oh2[0, :64] = 1.0
    oh2[1, 64:] = 1.0

    xts = [np.ascontiguousarray(x[b].T).astype(ml_dtypes.bfloat16) for b in range(4)]

    def fuse(Wslc, bslc):
        # W_fused[:, (h f)] = sum_d W.T[:, (h d)] pt[d, f]; bias likewise
        wT = Wslc.T.reshape(D, 8, HD)
        wf = np.einsum("ahd,df->ahf", wT, pt).reshape(D, 512)
        bf = np.einsum("hd,df->hf", bslc.reshape(8, HD), pt).reshape(512)
        return wf, bf

    in_maps = []
    for c in range(8):
        b, g = c // 2, c % 2
        wqs = W_qkv[g * 512 : (g + 1) * 512]
        wks = W_qkv[D + g * 512 : D + (g + 1) * 512]
        wvs = W_qkv[2 * D + g * 512 : 2 * D + (g + 1) * 512]
        bqs = b_qkv[g * 512 : (g + 1) * 512]
        bks = b_qkv[D + g * 512 : D + (g + 1) * 512]
        bvs = b_qkv[2 * D + g * 512 : 2 * D + (g + 1) * 512]
        wqp, bqp = fuse(wqs, bqs)
        wkp, bkp = fuse(wks, bks)
        bvb = np.empty((128, 4, 64), dtype=np.float32)
        bv_r = bvs.reshape(4, 2, 64)
        for p in range(4):
            bvb[0:64, p, :] = bv_r[p, 0][None, :]
            bvb[64:128, p, :] = bv_r[p, 1][None, :]
        in_maps.append(
            {
                "xt": xts[b],
                "wqp": np.ascontiguousarray(wqp).astype(ml_dtypes.bfloat16),
                "wkp": np.ascontiguousarray(wkp).astype(ml_dtypes.bfloat16),
                "wv": np.ascontiguousarray(wvs.T).astype(ml_dtypes.bfloat16),
                "wp": np.ascontiguousarray(
                    W_proj[:, g * 512 : (g + 1) * 512].T
                ).astype(ml_dtypes.bfloat16),
                "bqpe": np.ascontiguousarray(
                    (bqp + EPS).reshape(4, 128).T
                ).astype(np.float32),
                "bkpb": np.ascontiguousarray(
                    np.broadcast_to(bkp.reshape(1, 8, 64), (128, 8, 64))
                ).astype(np.float32),
                "bvb": bvb,
                "oh2": oh2.astype(ml_dtypes.bfloat16),
            }
        )
    return in_maps


def kernel(x, W_qkv, b_qkv, W_proj, b_proj, proj_mat):
    b_proj = np.asarray(b_proj, dtype=np.float32)
    in_maps = _host_inputs(x, W_qkv, b_qkv, W_proj, b_proj, proj_mat)
    nc = _get_nc()
    res = run_bass_kernel_spmd(nc, in_maps, core_ids=list(range(8)))
    final = np.empty((4, N, D), dtype=np.float32)
    for b in range(4):
        acc = res.results[2 * b]["out"] + res.results[2 * b + 1]["out"]
        final[b] = acc.T + b_proj[None, :]
    return final
